# revision 14
# baseline (speedup 1.0000x reference)
"""Contextual kNN similarity kernel for Trainium2, 8 NeuronCores.

For z [4096, 512] fp32 computes (matching reference.py's structure):
    d   = sq_i + sq_j - 2 z z^T
    kth = 10th smallest d per row (ties included via <=)
    M   = (d <= kth)                       [N, N] 0/1
    S1  = (M M^T) / rowsum(M)
    R   = M * M^T
    S2  = (S1 @ R^T) / rowsum(R)           (R symmetric)
    out = 0.5 (S2 + S2^T)

Distribution: row-shard over 8 cores (512 rows each, z replicated).
Each core: row block of -d (fp32 PE matmul mirroring the reference's
rounding), local top-10 via DVE max8/match_replace/max8, 0/1 mask in
bf16 (exact). PE-transpose mask -> AllGather (bf16). R rows = M .*
gathered-columns (exact), second AllGather. Then two big matmuls with
exact small-integer arithmetic (bf16 inputs, fp32 PSUM):
  Ct = (M M^T)[:, own]   and   W = sum_t C[t, i] R[t, j]
Both row-normalizations and the final 0.5 fold into one per-row scale
0.5/(rs1*rsR) applied on PSUM evacuation. Symmetrization: AllToAll of
the scaled S2, PE transpose, add. Host only preps/replicates inputs and
concatenates the 8 row-block outputs.
"""
import numpy as np

NC_CORES = 8
N = 4096
D = 512
B = N // NC_CORES        # 512 rows per core
P = 128
MC = B // P              # 4 m-chunks of own rows
NCH = N // 512           # 8 n-chunks
KC = D // P              # 4 contraction chunks for g
TC = N // P              # 32 t/k chunks
NEG_BIG = -3.0e38

_cache = {}


def _build():
    from concourse import bass, bacc, tile, mybir

    f32 = mybir.dt.float32
    bf16 = mybir.dt.bfloat16
    AL = mybir.AluOpType
    AX = mybir.AxisListType.X

    nc = bacc.Bacc(
        "TRN2",
        target_bir_lowering=False,
        debug=False,
        enable_asserts=False,
        num_devices=NC_CORES,
    )

    zt_in = nc.dram_tensor("zt", [D, N], f32, kind="ExternalInput").ap()
    zt_own_in = nc.dram_tensor("zt_own", [D, B], f32, kind="ExternalInput").ap()
    id32_in = nc.dram_tensor("id32", [P, P], f32, kind="ExternalInput").ap()
    id16_in = nc.dram_tensor("id16", [P, P], bf16, kind="ExternalInput").ap()
    out_ext = nc.dram_tensor("out", [B, N], f32, kind="ExternalOutput").ap()

    with tile.TileContext(nc) as tc:
        with tc.tile_pool(name="ps_mm", bufs=3, space="PSUM") as ps_mm, \
             tc.tile_pool(name="ps_tp", bufs=2, space="PSUM") as ps_tp, \
             tc.tile_pool(name="dram", bufs=1, space="DRAM") as dram, \
             tc.tile_pool(name="p_small", bufs=1) as p_small:

            pid = nc.sync.partition_id()

            ones128 = p_small.tile([P, P], f32, tag="ones128")
            nc.vector.memset(ones128[:], 1.0)
            id32 = p_small.tile([P, P], f32, tag="id32")
            nc.sync.dma_start(id32[:], id32_in[:])
            id16 = p_small.tile([P, P], bf16, tag="id16")
            nc.sync.dma_start(id16[:], id16_in[:])

            rs1 = [p_small.tile([P, 1], f32, tag=f"rs1_{m}", name=f"rs1_{m}")
                   for m in range(MC)]
            rsR = [p_small.tile([P, 1], f32, tag=f"rsR{m}", name=f"rsR{m}")
                   for m in range(MC)]

            # Overlapping lifetimes (mask A..C, mt B..D, ct D..E, s2 E..F):
            # pools must release in per-side LIFO order, so alternate the
            # chain between the left and right SBUF stacks.
            cm_mask = tc.tile_pool(name="p_mask", bufs=1)          # left
            p_mask = cm_mask.__enter__()
            mask_bf = [p_mask.tile([P, N], bf16, tag=f"mask{m}",
                                   name=f"mask{m}") for m in range(MC)]

            # ============ phase A: sq, g, negd, top-k, mask ============
            with tc.tile_pool(name="p_a", bufs=1) as p_a:
                zt_sb = p_a.tile([P, KC, N], f32, tag="zt")          # 64 KB/p
                nc.sync.dma_start(
                    zt_sb[:], zt_in.rearrange("(kc p) n -> p kc n", p=P))
                zt_own_sb = p_a.tile([P, KC, B], f32, tag="zt_own")
                nc.sync.dma_start(
                    zt_own_sb[:], zt_own_in.rearrange("(kc p) n -> p kc n", p=P))

                # --- sq broadcast over partitions (exact, fixed order)
                sq_bcast = p_a.tile([P, N], f32, tag="sq_bcast")     # 16 KB/p
                sq_own_cols = []
                with tc.tile_pool(name="p_zsq", bufs=1) as p_zsq:
                    for n in range(NCH):
                        zq = p_zsq.tile([P, KC, 512], f32, tag="zq", bufs=2)
                        for kc in range(KC):
                            nc.vector.tensor_mul(
                                zq[:, kc, :],
                                zt_sb[:, kc, n * 512:(n + 1) * 512],
                                zt_sb[:, kc, n * 512:(n + 1) * 512])
                        ps = ps_mm.tile([P, 512], f32, tag="mm")
                        for kc in range(KC):
                            nc.tensor.matmul(
                                ps[:], ones128[:], zq[:, kc, :],
                                start=(kc == 0), stop=(kc == KC - 1))
                        nc.scalar.copy(sq_bcast[:, n * 512:(n + 1) * 512],
                                       ps[:])

                    # own-rows sq -> per-partition [P,1], bitwise-equal values
                    zsq_own = p_zsq.tile([P, KC, B], f32, tag="zsq_own")
                    for kc in range(KC):
                        nc.vector.tensor_mul(
                            zsq_own[:, kc, :], zt_own_sb[:, kc, :],
                            zt_own_sb[:, kc, :])
                    for m in range(MC):
                        ps = ps_mm.tile([P, P], f32, tag="mm")
                        for kc in range(KC):
                            nc.tensor.matmul(
                                ps[:], ones128[:],
                                zsq_own[:, kc, m * P:(m + 1) * P],
                                start=(kc == 0), stop=(kc == KC - 1))
                        sq_bc_m = p_zsq.tile([P, P], f32, tag="sq_bc_m",
                                             bufs=2)
                        nc.scalar.copy(sq_bc_m[:], ps[:])
                        psT = ps_tp.tile([P, P], f32, tag="tp")
                        nc.tensor.transpose(psT[:], sq_bc_m[:], id32[:])
                        sqc = p_small.tile([P, 1], f32, tag=f"sq_own{m}",
                                           name=f"sq_own{m}")
                        nc.vector.tensor_copy(sqc[:], psT[:, 0:1])
                        sq_own_cols.append(sqc)

                # lhsT for g: 2 * zt_own (exact)
                zt_own2 = p_a.tile([P, KC, B], f32, tag="zt_own2")
                for kc in range(KC):
                    nc.vector.tensor_scalar_mul(
                        zt_own2[:, kc, :], zt_own_sb[:, kc, :], 2.0)

                # --- per m-chunk: negd = fl(2g - fl(sq_i+sq_j)), topk, mask
                with tc.tile_pool(name="p_tk", bufs=1) as p_tk:
                    for m in range(MC):
                        negd = p_tk.tile([P, N], f32, tag="negd", bufs=2)
                        for n in range(NCH):
                            ps = ps_mm.tile([P, 512], f32, tag="mm")
                            for kc in range(KC):
                                nc.tensor.matmul(
                                    ps[:],
                                    zt_own2[:, kc, m * P:(m + 1) * P],
                                    zt_sb[:, kc, n * 512:(n + 1) * 512],
                                    start=(kc == 0), stop=(kc == KC - 1))
                            s_chunk = p_tk.tile([P, 512], f32, tag="s_chunk",
                                                bufs=2)
                            nc.vector.tensor_scalar(
                                out=s_chunk[:],
                                in0=sq_bcast[:, n * 512:(n + 1) * 512],
                                scalar1=sq_own_cols[m][:], scalar2=None,
                                op0=AL.add)
                            nc.vector.tensor_tensor(
                                out=negd[:, n * 512:(n + 1) * 512], in0=ps[:],
                                in1=s_chunk[:], op=AL.subtract)

                        mx1 = p_tk.tile([P, 8], f32, tag="mx1", bufs=2)
                        nc.vector.max(mx1[:], negd[:])
                        msk = p_tk.tile([P, N], f32, tag="msk", bufs=1)
                        nc.vector.match_replace(msk[:], mx1[:], negd[:],
                                                NEG_BIG)
                        mx2 = p_tk.tile([P, 8], f32, tag="mx2", bufs=2)
                        nc.vector.max(mx2[:], msk[:])
                        kth = p_tk.tile([P, 1], f32, tag="kth", bufs=2)
                        nc.vector.tensor_copy(kth[:], mx2[:, 1:2])

                        nc.vector.tensor_scalar(
                            out=mask_bf[m][:], in0=negd[:], scalar1=kth[:],
                            scalar2=None, op0=AL.is_ge)
                        nc.vector.reduce_sum(rs1[m][:], mask_bf[m][:],
                                             axis=AX)

            # ============ phase B: transpose mask, AllGather #1 ============
            # ag1 layout: per core [MC own-col sub-blocks][N k-rows][128],
            # so a Ct weight slab (all k for 128 t's) is 1 contiguous MB.
            cm_mt = tc.tile_pool(name="p_mt", bufs=1, side="right")
            p_mt = cm_mt.__enter__()
            mt_sb = p_mt.tile([P, TC, B], bf16, tag="mt")            # 32 KB/p
            for m in range(MC):
                for a4 in range(TC // 4):
                    psT = ps_tp.tile([P, 512], bf16, tag="tp")
                    for j in range(4):
                        a = a4 * 4 + j
                        nc.tensor.transpose(
                            psT[:, j * P:(j + 1) * P],
                            mask_bf[m][:, a * P:(a + 1) * P], id16[:])
                    nc.scalar.copy(
                        mt_sb[:, a4 * 4:(a4 + 1) * 4, m * P:(m + 1) * P],
                        psT.rearrange("p (j q) -> p j q", j=4))

            ag1_in = dram.tile([MC * N, P], bf16)
            for cb in range(MC):
                nc.sync.dma_start(
                    ag1_in[cb * N:(cb + 1) * N, :].rearrange(
                        "(a p) q -> p a q", p=P),
                    mt_sb[:, :, cb * P:(cb + 1) * P])
            ag1_out = dram.tile([NC_CORES * MC * N, P], bf16,
                                addr_space="Shared")
            nc.gpsimd.collective_compute(
                "AllGather", AL.bypass,
                replica_groups=[list(range(NC_CORES))],
                ins=[ag1_in[:].opt()], outs=[ag1_out[:].opt()],
            )

            # ============ phase C: R rows, AllGather #2 ============
            # X[q, j] = M[j, own q] = Mt[own row, j]; Mt row (pid*B+m*128+q),
            # col j=(r*B+c): ag1_out[(r*MC + c//128)*N + row, c%128]
            ag2_in = dram.tile([NCH * B, 512], bf16)
            with tc.tile_pool(name="p_r", bufs=1) as p_r:
                for m in range(MC):
                    rbf = p_r.tile([P, N], bf16, tag="rbf", bufs=2)
                    for r in range(NC_CORES):
                        xt = p_r.tile([P, MC, P], bf16, tag="xt", bufs=3)
                        for cb in range(MC):
                            nc.sync.dma_start(
                                xt[:, cb, :],
                                ag1_out[bass.ds(
                                    (r * MC + cb) * N + pid * B + m * P,
                                    P), :])
                        nc.vector.tensor_tensor(
                            out=rbf[:, r * B:(r + 1) * B],
                            in0=mask_bf[m][:, r * B:(r + 1) * B],
                            in1=xt.rearrange("p c q -> p (c q)"),
                            op=AL.mult)
                    nc.vector.reduce_sum(rsR[m][:], rbf[:], axis=AX)
                    # store in [jb][i][512] blocks for contiguous W rhs slabs
                    nc.sync.dma_start(
                        ag2_in.rearrange("(jb i) j -> i jb j", jb=NCH)[
                            m * P:(m + 1) * P, :, :],
                        rbf.rearrange("p (jb j) -> p jb j", jb=NCH))
            ag2_out = dram.tile([NC_CORES * NCH * B, 512], bf16,
                                addr_space="Shared")
            nc.gpsimd.collective_compute(
                "AllGather", AL.bypass,
                replica_groups=[list(range(NC_CORES))],
                ins=[ag2_in[:].opt()], outs=[ag2_out[:].opt()],
            )
            cm_mask.__exit__(None, None, None)   # masks dead

            # scale = 0.5 / (rs1 * rsR)
            halfrec = []
            for m in range(MC):
                prod = p_small.tile([P, 1], f32, tag=f"prod{m}",
                                    name=f"prod{m}")
                nc.vector.tensor_tensor(out=prod[:], in0=rs1[m][:],
                                        in1=rsR[m][:], op=AL.mult)
                rec = p_small.tile([P, 1], f32, tag=f"rec{m}",
                                   name=f"rec{m}")
                nc.vector.reciprocal(rec[:], prod[:])
                hr = p_small.tile([P, 1], f32, tag=f"hr{m}", name=f"hr{m}")
                nc.vector.tensor_scalar_mul(hr[:], rec[:], 0.5)
                halfrec.append(hr)

            # ============ phase D: Ct = (M M^T)[:, own] ============
            cm_ct = tc.tile_pool(name="p_ct", bufs=1)              # left
            p_ct = cm_ct.__enter__()
            ct_sb = p_ct.tile([P, TC, B], bf16, tag="ct")            # 32 KB/p
            with tc.tile_pool(name="p_lh", bufs=1) as p_lh:
                for t in range(TC):
                    lh = p_lh.tile([P, TC, P], bf16, tag="lh", bufs=3)
                    nc.sync.dma_start(
                        lh[:],
                        ag1_out[(t // MC) * MC * N + (t % MC) * N:
                                (t // MC) * MC * N + (t % MC + 1) * N, :]
                        .rearrange("(a p) q -> p a q", p=P))
                    ps = ps_mm.tile([P, B], f32, tag="mm")
                    for kc in range(TC):
                        nc.tensor.matmul(
                            ps[:], lh[:, kc, :], mt_sb[:, kc, :],
                            start=(kc == 0), stop=(kc == TC - 1))
                    nc.scalar.copy(ct_sb[:, t, :], ps[:])
            cm_mt.__exit__(None, None, None)     # mt dead

            # ============ phase E: S2 rows = scale * (Ct^T @ R) ============
            cm_s2 = tc.tile_pool(name="p_s2", bufs=1, side="right")
            p_s2 = cm_s2.__enter__()
            sim2 = [p_s2.tile([P, N], f32, tag=f"sim2_{m}", name=f"sim2_{m}")
                    for m in range(MC)]
            tkc = B // P
            with tc.tile_pool(name="p_w", bufs=1) as p_w:
                for n in range(NCH):
                    rsl = p_w.tile([P, NC_CORES, tkc, 512], bf16,
                                   tag="rsl", bufs=2)
                    for r in range(NC_CORES):
                        nc.sync.dma_start(
                            rsl[:, r, :, :],
                            ag2_out.rearrange(
                                "(r jb i) j -> jb r i j", r=NC_CORES,
                                jb=NCH)[n, r, :, :]
                            .rearrange("(tk p) j -> p tk j", p=P))
                    for m in range(MC):
                        ps = ps_mm.tile([P, 512], f32, tag="mm")
                        for t in range(TC):
                            nc.tensor.matmul(
                                ps[:], ct_sb[:, t, m * P:(m + 1) * P],
                                rsl[:, t // tkc, t % tkc, :],
                                start=(t == 0), stop=(t == TC - 1))
                        nc.scalar.activation(
                            sim2[m][:, n * 512:(n + 1) * 512], ps[:],
                            mybir.ActivationFunctionType.Copy,
                            scale=halfrec[m][:])
            cm_ct.__exit__(None, None, None)     # ct dead

            # ============ phase F: AllToAll + transpose + add ============
            a2a_in = dram.tile([N, B], f32)
            for m in range(MC):
                nc.sync.dma_start(
                    a2a_in.rearrange("(r i) j -> i r j", r=NC_CORES)[
                        m * P:(m + 1) * P, :, :],
                    sim2[m].rearrange("p (r j) -> p r j", r=NC_CORES))
            a2a_out = dram.tile([N, B], f32)
            nc.gpsimd.collective_compute(
                "AllToAll", AL.bypass,
                replica_groups=[list(range(NC_CORES))],
                ins=[a2a_in[:].opt()], outs=[a2a_out[:].opt()],
            )

            with tc.tile_pool(name="p_f", bufs=1) as p_f:
                for a4 in range(TC // 4):
                    recvs = []
                    for j in range(4):
                        a = a4 * 4 + j
                        rv = p_f.tile([P, B], f32, tag="rv", bufs=8)
                        nc.sync.dma_start(rv[:], a2a_out[a * P:(a + 1) * P, :])
                        recvs.append(rv)
                    for m in range(MC):
                        ps = ps_tp.tile([P, 512], f32, tag="ps_f", bufs=2)
                        for j in range(4):
                            nc.tensor.transpose(
                                ps[:, j * P:(j + 1) * P],
                                recvs[j][:, m * P:(m + 1) * P], id32[:])
                        ob = p_f.tile([P, 512], f32, tag="ob", bufs=3)
                        nc.vector.tensor_tensor(
                            out=ob[:], in0=ps[:],
                            in1=sim2[m][:, a4 * 512:(a4 + 1) * 512],
                            op=AL.add)
                        nc.sync.dma_start(
                            out_ext[m * P:(m + 1) * P,
                                    a4 * 512:(a4 + 1) * 512],
                            ob[:])
            cm_s2.__exit__(None, None, None)

    nc.compile()
    return nc


def _get_nc():
    if "nc" not in _cache:
        _cache["nc"] = _build()
    return _cache["nc"]


def _bf16(x):
    try:
        import ml_dtypes
        return x.astype(ml_dtypes.bfloat16)
    except ImportError:
        from concourse import mybir
        return x.astype(mybir.dt.np(mybir.dt.bfloat16))


def kernel(z, _profile=False):
    from concourse import bass_utils

    z = np.ascontiguousarray(np.asarray(z, dtype=np.float32))
    assert z.shape == (N, D), z.shape
    zT = np.ascontiguousarray(z.T)
    id32 = np.eye(P, dtype=np.float32)
    id16 = _bf16(np.eye(P, dtype=np.float32))

    nc = _get_nc()
    in_maps = []
    for c in range(NC_CORES):
        in_maps.append({
            "zt": zT,
            "zt_own": np.ascontiguousarray(zT[:, c * B:(c + 1) * B]),
            "id32": id32,
            "id16": id16,
        })
    res = bass_utils.run_bass_kernel_spmd(
        nc, in_maps, core_ids=list(range(NC_CORES)), trace=_profile)
    out = np.concatenate(
        [res.results[c]["out"] for c in range(NC_CORES)], axis=0)
    if _profile:
        return out, res
    return out


# revision 17
# speedup vs baseline: 1.0440x; 1.0440x over previous
"""Contextual kNN similarity kernel for Trainium2, 8 NeuronCores.

For z [4096, 512] fp32 computes (matching reference.py's structure):
    d   = sq_i + sq_j - 2 z z^T
    kth = 10th smallest d per row (ties included via <=)
    M   = (d <= kth)                       [N, N] 0/1
    S1  = (M M^T) / rowsum(M)
    R   = M * M^T
    S2  = (S1 @ R^T) / rowsum(R)           (R symmetric)
    out = 0.5 (S2 + S2^T)

Distribution: row-shard over 8 cores (512 rows each, z replicated).
Each core: row block of -d (fp32 PE matmul mirroring the reference's
rounding), local top-10 via DVE max8/match_replace/max8, 0/1 mask in
bf16 (exact). PE-transpose mask -> AllGather (bf16). R rows = M .*
gathered-columns (exact), second AllGather. Then two big matmuls with
exact small-integer arithmetic (bf16 inputs, fp32 PSUM):
  Ct = (M M^T)[:, own]   and   W = sum_t C[t, i] R[t, j]
Both row-normalizations and the final 0.5 fold into one per-row scale
0.5/(rs1*rsR) applied on PSUM evacuation. Symmetrization: AllToAll of
the scaled S2, PE transpose, add. Host only preps/replicates inputs and
concatenates the 8 row-block outputs.
"""
import numpy as np

NC_CORES = 8
N = 4096
D = 512
B = N // NC_CORES        # 512 rows per core
P = 128
MC = B // P              # 4 m-chunks of own rows
NCH = N // 512           # 8 n-chunks
KC = D // P              # 4 contraction chunks for g
TC = N // P              # 32 t/k chunks
NEG_BIG = -3.0e38

_cache = {}


def _build():
    from concourse import bass, bacc, tile, mybir

    f32 = mybir.dt.float32
    bf16 = mybir.dt.bfloat16
    AL = mybir.AluOpType
    AX = mybir.AxisListType.X

    nc = bacc.Bacc(
        "TRN2",
        target_bir_lowering=False,
        debug=False,
        enable_asserts=False,
        num_devices=NC_CORES,
    )

    zt_in = nc.dram_tensor("zt", [D, N], f32, kind="ExternalInput").ap()
    zt_own_in = nc.dram_tensor("zt_own", [D, B], f32, kind="ExternalInput").ap()
    id32_in = nc.dram_tensor("id32", [P, P], f32, kind="ExternalInput").ap()
    id16_in = nc.dram_tensor("id16", [P, P], bf16, kind="ExternalInput").ap()
    out_ext = nc.dram_tensor("out", [B, N], f32, kind="ExternalOutput").ap()

    with tile.TileContext(nc) as tc:
        with tc.tile_pool(name="ps_mm", bufs=3, space="PSUM") as ps_mm, \
             tc.tile_pool(name="ps_tp", bufs=2, space="PSUM") as ps_tp, \
             tc.tile_pool(name="dram", bufs=1, space="DRAM") as dram, \
             tc.tile_pool(name="p_small", bufs=1) as p_small:

            pid = nc.sync.partition_id()

            ones128 = p_small.tile([P, P], f32, tag="ones128")
            nc.vector.memset(ones128[:], 1.0)
            id32 = p_small.tile([P, P], f32, tag="id32")
            nc.sync.dma_start(id32[:], id32_in[:])
            id16 = p_small.tile([P, P], bf16, tag="id16")
            nc.sync.dma_start(id16[:], id16_in[:])

            rs1 = [p_small.tile([P, 1], f32, tag=f"rs1_{m}", name=f"rs1_{m}")
                   for m in range(MC)]
            rsR = [p_small.tile([P, 1], f32, tag=f"rsR{m}", name=f"rsR{m}")
                   for m in range(MC)]

            # Overlapping lifetimes (mask A..C, mt B..D, ct D..E, s2 E..F):
            # pools must release in per-side LIFO order, so alternate the
            # chain between the left and right SBUF stacks.
            cm_mask = tc.tile_pool(name="p_mask", bufs=1)          # left
            p_mask = cm_mask.__enter__()
            mask_bf = [p_mask.tile([P, N], bf16, tag=f"mask{m}",
                                   name=f"mask{m}") for m in range(MC)]

            # ============ phase A: sq, g, negd, top-k, mask ============
            with tc.tile_pool(name="p_a", bufs=1) as p_a:
                zt_sb = p_a.tile([P, KC, N], f32, tag="zt")          # 64 KB/p
                nc.sync.dma_start(
                    zt_sb[:], zt_in.rearrange("(kc p) n -> p kc n", p=P))
                zt_own_sb = p_a.tile([P, KC, B], f32, tag="zt_own")
                nc.sync.dma_start(
                    zt_own_sb[:], zt_own_in.rearrange("(kc p) n -> p kc n", p=P))

                # --- sq broadcast over partitions (exact, fixed order)
                sq_bcast = p_a.tile([P, N], f32, tag="sq_bcast")     # 16 KB/p
                sq_own_cols = []
                with tc.tile_pool(name="p_zsq", bufs=1) as p_zsq:
                    for n in range(NCH):
                        zq = p_zsq.tile([P, KC, 512], f32, tag="zq", bufs=2)
                        for kc in range(KC):
                            nc.vector.tensor_mul(
                                zq[:, kc, :],
                                zt_sb[:, kc, n * 512:(n + 1) * 512],
                                zt_sb[:, kc, n * 512:(n + 1) * 512])
                        ps = ps_mm.tile([P, 512], f32, tag="mm")
                        for kc in range(KC):
                            nc.tensor.matmul(
                                ps[:], ones128[:], zq[:, kc, :],
                                start=(kc == 0), stop=(kc == KC - 1))
                        nc.scalar.copy(sq_bcast[:, n * 512:(n + 1) * 512],
                                       ps[:])

                    # own-rows sq -> per-partition [P,1], bitwise-equal values
                    zsq_own = p_zsq.tile([P, KC, B], f32, tag="zsq_own")
                    for kc in range(KC):
                        nc.vector.tensor_mul(
                            zsq_own[:, kc, :], zt_own_sb[:, kc, :],
                            zt_own_sb[:, kc, :])
                    for m in range(MC):
                        ps = ps_mm.tile([P, P], f32, tag="mm")
                        for kc in range(KC):
                            nc.tensor.matmul(
                                ps[:], ones128[:],
                                zsq_own[:, kc, m * P:(m + 1) * P],
                                start=(kc == 0), stop=(kc == KC - 1))
                        sq_bc_m = p_zsq.tile([P, P], f32, tag="sq_bc_m",
                                             bufs=2)
                        nc.scalar.copy(sq_bc_m[:], ps[:])
                        psT = ps_tp.tile([P, P], f32, tag="tp")
                        nc.tensor.transpose(psT[:], sq_bc_m[:], id32[:])
                        sqc = p_small.tile([P, 1], f32, tag=f"sq_own{m}",
                                           name=f"sq_own{m}")
                        nc.vector.tensor_copy(sqc[:], psT[:, 0:1])
                        sq_own_cols.append(sqc)

                # lhsT for g: 2 * zt_own (exact)
                zt_own2 = p_a.tile([P, KC, B], f32, tag="zt_own2")
                for kc in range(KC):
                    nc.vector.tensor_scalar_mul(
                        zt_own2[:, kc, :], zt_own_sb[:, kc, :], 2.0)

                # --- per m-chunk: negd = fl(2g - fl(sq_i+sq_j)), topk, mask
                with tc.tile_pool(name="p_tk", bufs=1) as p_tk:
                    for m in range(MC):
                        negd = p_tk.tile([P, N], f32, tag="negd", bufs=2)
                        for n in range(NCH):
                            ps = ps_mm.tile([P, 512], f32, tag="mm")
                            for kc in range(KC):
                                nc.tensor.matmul(
                                    ps[:],
                                    zt_own2[:, kc, m * P:(m + 1) * P],
                                    zt_sb[:, kc, n * 512:(n + 1) * 512],
                                    start=(kc == 0), stop=(kc == KC - 1))
                            s_chunk = p_tk.tile([P, 512], f32, tag="s_chunk",
                                                bufs=2)
                            nc.vector.tensor_scalar(
                                out=s_chunk[:],
                                in0=sq_bcast[:, n * 512:(n + 1) * 512],
                                scalar1=sq_own_cols[m][:], scalar2=None,
                                op0=AL.add)
                            nc.vector.tensor_tensor(
                                out=negd[:, n * 512:(n + 1) * 512], in0=ps[:],
                                in1=s_chunk[:], op=AL.subtract)

                        mx1 = p_tk.tile([P, 8], f32, tag="mx1", bufs=2)
                        nc.vector.max(mx1[:], negd[:])
                        msk = p_tk.tile([P, N], f32, tag="msk", bufs=1)
                        nc.vector.match_replace(msk[:], mx1[:], negd[:],
                                                NEG_BIG)
                        mx2 = p_tk.tile([P, 8], f32, tag="mx2", bufs=2)
                        nc.vector.max(mx2[:], msk[:])
                        kth = p_tk.tile([P, 1], f32, tag="kth", bufs=2)
                        nc.vector.tensor_copy(kth[:], mx2[:, 1:2])

                        nc.vector.tensor_scalar(
                            out=mask_bf[m][:], in0=negd[:], scalar1=kth[:],
                            scalar2=None, op0=AL.is_ge)
                        nc.vector.reduce_sum(rs1[m][:], mask_bf[m][:],
                                             axis=AX)

            # ============ phase B: transpose mask, AllGather #1 ============
            # ag1 layout: per core [MC own-col sub-blocks][N k-rows][128],
            # so a Ct weight slab (all k for 128 t's) is 1 contiguous MB.
            cm_mt = tc.tile_pool(name="p_mt", bufs=1, side="right")
            p_mt = cm_mt.__enter__()
            mt_sb = p_mt.tile([P, TC, B], bf16, tag="mt")            # 32 KB/p
            # One AllGather chunk per own-column sub-block cb (== mask m):
            # launches as soon as topk(m) is done, overlapping phase A.
            # Gathered layout: [cb][r][N][P].
            ag1_in = dram.tile([MC * N, P], bf16)
            ag1_outs = [dram.tile([NC_CORES * N, P], bf16,
                                  addr_space="Shared", name=f"ag1o{cb}")
                        for cb in range(MC)]
            for m in range(MC):
                for a4 in range(TC // 4):
                    psT = ps_tp.tile([P, 512], bf16, tag="tp")
                    for j in range(4):
                        a = a4 * 4 + j
                        nc.tensor.transpose(
                            psT[:, j * P:(j + 1) * P],
                            mask_bf[m][:, a * P:(a + 1) * P], id16[:])
                    nc.scalar.copy(
                        mt_sb[:, a4 * 4:(a4 + 1) * 4, m * P:(m + 1) * P],
                        psT.rearrange("p (j q) -> p j q", j=4))
                nc.sync.dma_start(
                    ag1_in[m * N:(m + 1) * N, :].rearrange(
                        "(a p) q -> p a q", p=P),
                    mt_sb[:, :, m * P:(m + 1) * P])
                nc.gpsimd.collective_compute(
                    "AllGather", AL.bypass,
                    replica_groups=[list(range(NC_CORES))],
                    ins=[ag1_in[m * N:(m + 1) * N, :].opt()],
                    outs=[ag1_outs[m][:].opt()],
                )

            # ============ phase C: R rows, AllGather #2 ============
            # X[q, j] = M[j, own q] = Mt[own row, j]; Mt row (pid*B+m*128+q),
            # col j=(r*B+c): ag1_out[(r*MC + c//128)*N + row, c%128]
            # Gathered R layout: [m][r][jb][128][512]; one AllGather chunk
            # per m, launched right after R(m) -- overlaps with R(m+1)/Ct.
            # Gathered R layout: per-m tensors [r][jb][128][512]; one
            # AllGather chunk per m, launched right after R(m).
            ag2_in = dram.tile([MC * NCH * P, 512], bf16)
            ag2_outs = [dram.tile([NC_CORES * NCH * P, 512], bf16,
                                  addr_space="Shared", name=f"ag2o{m}")
                        for m in range(MC)]
            with tc.tile_pool(name="p_r", bufs=1) as p_r:
                for m in range(MC):
                    rbf = p_r.tile([P, N], bf16, tag="rbf", bufs=2)
                    # X for all sources r, own rows m: one DMA per cb
                    xt = p_r.tile([P, MC, NC_CORES, P], bf16, tag="xt",
                                  bufs=2)
                    for cb in range(MC):
                        nc.sync.dma_start(
                            xt[:, cb, :, :],
                            ag1_outs[cb].rearrange(
                                "(r a) q -> a r q", r=NC_CORES)[
                                bass.ds(pid * B + m * P, P), :, :])
                    for r in range(NC_CORES):
                        nc.vector.tensor_tensor(
                            out=rbf[:, r * B:(r + 1) * B].rearrange(
                                "p (c q) -> p c q", c=MC),
                            in0=mask_bf[m][:, r * B:(r + 1) * B].rearrange(
                                "p (c q) -> p c q", c=MC),
                            in1=xt[:, :, r, :],
                            op=AL.mult)
                    nc.vector.reduce_sum(rsR[m][:], rbf[:], axis=AX)
                    nc.sync.dma_start(
                        ag2_in[m * NCH * P:(m + 1) * NCH * P, :].rearrange(
                            "(jb q) j -> q jb j", jb=NCH),
                        rbf.rearrange("p (jb j) -> p jb j", jb=NCH))
                    nc.gpsimd.collective_compute(
                        "AllGather", AL.bypass,
                        replica_groups=[list(range(NC_CORES))],
                        ins=[ag2_in[m * NCH * P:(m + 1) * NCH * P, :].opt()],
                        outs=[ag2_outs[m][:].opt()],
                    )
            cm_mask.__exit__(None, None, None)   # masks dead

            # scale = 0.5 / (rs1 * rsR)
            halfrec = []
            for m in range(MC):
                prod = p_small.tile([P, 1], f32, tag=f"prod{m}",
                                    name=f"prod{m}")
                nc.vector.tensor_tensor(out=prod[:], in0=rs1[m][:],
                                        in1=rsR[m][:], op=AL.mult)
                rec = p_small.tile([P, 1], f32, tag=f"rec{m}",
                                   name=f"rec{m}")
                nc.vector.reciprocal(rec[:], prod[:])
                hr = p_small.tile([P, 1], f32, tag=f"hr{m}", name=f"hr{m}")
                nc.vector.tensor_scalar_mul(hr[:], rec[:], 0.5)
                halfrec.append(hr)

            # ============ phase D: Ct = (M M^T)[:, own] ============
            cm_ct = tc.tile_pool(name="p_ct", bufs=1)              # left
            p_ct = cm_ct.__enter__()
            ct_sb = p_ct.tile([P, TC, B], bf16, tag="ct")            # 32 KB/p
            with tc.tile_pool(name="p_lh", bufs=1) as p_lh:
                for t in range(TC):
                    # t covers Mt cols [t*128, +128): source core r = t//MC,
                    # sub-block cb = t%MC -> gathered block (cb, r).
                    lh = p_lh.tile([P, TC, P], bf16, tag="lh", bufs=3)
                    nc.scalar.dma_start(
                        lh[:],
                        ag1_outs[t % MC][(t // MC) * N:(t // MC + 1) * N, :]
                        .rearrange("(a p) q -> p a q", p=P))
                    ps = ps_mm.tile([P, B], f32, tag="mm")
                    for kc in range(TC):
                        nc.tensor.matmul(
                            ps[:], lh[:, kc, :], mt_sb[:, kc, :],
                            start=(kc == 0), stop=(kc == TC - 1))
                    nc.scalar.copy(ct_sb[:, t, :], ps[:])
            cm_mt.__exit__(None, None, None)     # mt dead

            # ============ phase E: S2 rows = scale * (Ct^T @ R) ============
            cm_s2 = tc.tile_pool(name="p_s2", bufs=1, side="right")
            p_s2 = cm_s2.__enter__()
            sim2 = [p_s2.tile([P, N], f32, tag=f"sim2_{m}", name=f"sim2_{m}")
                    for m in range(MC)]
            tkc = B // P
            with tc.tile_pool(name="p_w", bufs=1) as p_w:
                for n in range(NCH):
                    rsl = p_w.tile([P, NC_CORES, tkc, 512], bf16,
                                   tag="rsl", bufs=2)
                    for m in range(tkc):
                        nc.sync.dma_start(
                            rsl[:, :, m, :],
                            ag2_outs[m].rearrange(
                                "(r jb q) j -> r jb q j", r=NC_CORES,
                                jb=NCH)[:, n, :, :]
                            .rearrange("r q j -> q r j"))
                    for m in range(MC):
                        ps = ps_mm.tile([P, 512], f32, tag="mm")
                        for t in range(TC):
                            nc.tensor.matmul(
                                ps[:], ct_sb[:, t, m * P:(m + 1) * P],
                                rsl[:, t // tkc, t % tkc, :],
                                start=(t == 0), stop=(t == TC - 1))
                        nc.scalar.activation(
                            sim2[m][:, n * 512:(n + 1) * 512], ps[:],
                            mybir.ActivationFunctionType.Copy,
                            scale=halfrec[m][:])
            cm_ct.__exit__(None, None, None)     # ct dead

            # ============ phase F: AllToAll + transpose + add ============
            a2a_in = dram.tile([N, B], f32)
            for m in range(MC):
                nc.sync.dma_start(
                    a2a_in.rearrange("(r i) j -> i r j", r=NC_CORES)[
                        m * P:(m + 1) * P, :, :],
                    sim2[m].rearrange("p (r j) -> p r j", r=NC_CORES))
            a2a_out = dram.tile([N, B], f32)
            nc.gpsimd.collective_compute(
                "AllToAll", AL.bypass,
                replica_groups=[list(range(NC_CORES))],
                ins=[a2a_in[:].opt()], outs=[a2a_out[:].opt()],
            )

            with tc.tile_pool(name="p_f", bufs=1) as p_f:
                for a4 in range(TC // 4):
                    recvs = []
                    for j in range(4):
                        a = a4 * 4 + j
                        rv = p_f.tile([P, B], f32, tag="rv", bufs=8)
                        nc.sync.dma_start(rv[:], a2a_out[a * P:(a + 1) * P, :])
                        recvs.append(rv)
                    for m in range(MC):
                        ps = ps_tp.tile([P, 512], f32, tag="ps_f", bufs=2)
                        for j in range(4):
                            nc.tensor.transpose(
                                ps[:, j * P:(j + 1) * P],
                                recvs[j][:, m * P:(m + 1) * P], id32[:])
                        ob = p_f.tile([P, 512], f32, tag="ob", bufs=3)
                        nc.vector.tensor_tensor(
                            out=ob[:], in0=ps[:],
                            in1=sim2[m][:, a4 * 512:(a4 + 1) * 512],
                            op=AL.add)
                        nc.sync.dma_start(
                            out_ext[m * P:(m + 1) * P,
                                    a4 * 512:(a4 + 1) * 512],
                            ob[:])
            cm_s2.__exit__(None, None, None)

    nc.compile()
    return nc


def _get_nc():
    if "nc" not in _cache:
        _cache["nc"] = _build()
    return _cache["nc"]


def _bf16(x):
    try:
        import ml_dtypes
        return x.astype(ml_dtypes.bfloat16)
    except ImportError:
        from concourse import mybir
        return x.astype(mybir.dt.np(mybir.dt.bfloat16))


def kernel(z, _profile=False):
    from concourse import bass_utils

    z = np.ascontiguousarray(np.asarray(z, dtype=np.float32))
    assert z.shape == (N, D), z.shape
    zT = np.ascontiguousarray(z.T)
    id32 = np.eye(P, dtype=np.float32)
    id16 = _bf16(np.eye(P, dtype=np.float32))

    nc = _get_nc()
    in_maps = []
    for c in range(NC_CORES):
        in_maps.append({
            "zt": zT,
            "zt_own": np.ascontiguousarray(zT[:, c * B:(c + 1) * B]),
            "id32": id32,
            "id16": id16,
        })
    res = bass_utils.run_bass_kernel_spmd(
        nc, in_maps, core_ids=list(range(NC_CORES)), trace=_profile)
    out = np.concatenate(
        [res.results[c]["out"] for c in range(NC_CORES)], axis=0)
    if _profile:
        return out, res
    return out


# revision 18
# speedup vs baseline: 1.1020x; 1.0555x over previous
"""Contextual kNN similarity kernel for Trainium2, 8 NeuronCores.

For z [4096, 512] fp32 computes (matching reference.py's structure):
    d   = sq_i + sq_j - 2 z z^T
    kth = 10th smallest d per row (ties included via <=)
    M   = (d <= kth)                       [N, N] 0/1
    S1  = (M M^T) / rowsum(M)
    R   = M * M^T
    S2  = (S1 @ R^T) / rowsum(R)           (R symmetric)
    out = 0.5 (S2 + S2^T)

Distribution: row-shard over 8 cores (512 rows each, z replicated).
Each core: row block of -d (fp32 PE matmul mirroring the reference's
rounding), local top-10 via DVE max8/match_replace/max8, 0/1 mask in
bf16 (exact). PE-transpose mask -> AllGather (bf16). R rows = M .*
gathered-columns (exact), second AllGather. Then two big matmuls with
exact small-integer arithmetic (bf16 inputs, fp32 PSUM):
  Ct = (M M^T)[:, own]   and   W = sum_t C[t, i] R[t, j]
Both row-normalizations and the final 0.5 fold into one per-row scale
0.5/(rs1*rsR) applied on PSUM evacuation. Symmetrization: AllToAll of
the scaled S2, PE transpose, add. Host only preps/replicates inputs and
concatenates the 8 row-block outputs.
"""
import numpy as np

NC_CORES = 8
N = 4096
D = 512
B = N // NC_CORES        # 512 rows per core
P = 128
MC = B // P              # 4 m-chunks of own rows
NCH = N // 512           # 8 n-chunks
KC = D // P              # 4 contraction chunks for g
TC = N // P              # 32 t/k chunks
NEG_BIG = -3.0e38

_cache = {}


def _build():
    from concourse import bass, bacc, tile, mybir

    f32 = mybir.dt.float32
    bf16 = mybir.dt.bfloat16
    AL = mybir.AluOpType
    AX = mybir.AxisListType.X

    nc = bacc.Bacc(
        "TRN2",
        target_bir_lowering=False,
        debug=False,
        enable_asserts=False,
        num_devices=NC_CORES,
    )

    zt_in = nc.dram_tensor("zt", [D, N], f32, kind="ExternalInput").ap()
    zt_own_in = nc.dram_tensor("zt_own", [D, B], f32, kind="ExternalInput").ap()
    id32_in = nc.dram_tensor("id32", [P, P], f32, kind="ExternalInput").ap()
    id16_in = nc.dram_tensor("id16", [P, P], bf16, kind="ExternalInput").ap()
    out_ext = nc.dram_tensor("out", [B, N], f32, kind="ExternalOutput").ap()

    with tile.TileContext(nc) as tc:
        with tc.tile_pool(name="ps_mm", bufs=3, space="PSUM") as ps_mm, \
             tc.tile_pool(name="ps_tp", bufs=2, space="PSUM") as ps_tp, \
             tc.tile_pool(name="dram", bufs=1, space="DRAM") as dram, \
             tc.tile_pool(name="p_small", bufs=1) as p_small:

            pid = nc.sync.partition_id()

            ones128 = p_small.tile([P, P], f32, tag="ones128")
            nc.vector.memset(ones128[:], 1.0)
            id32 = p_small.tile([P, P], f32, tag="id32")
            nc.sync.dma_start(id32[:], id32_in[:])
            id16 = p_small.tile([P, P], bf16, tag="id16")
            nc.sync.dma_start(id16[:], id16_in[:])

            rs1 = [p_small.tile([P, 1], f32, tag=f"rs1_{m}", name=f"rs1_{m}")
                   for m in range(MC)]
            rsR = [p_small.tile([P, 1], f32, tag=f"rsR{m}", name=f"rsR{m}")
                   for m in range(MC)]

            # Overlapping lifetimes (mask A..C, mt B..D, ct D..E, s2 E..F):
            # pools must release in per-side LIFO order, so alternate the
            # chain between the left and right SBUF stacks.
            cm_mask = tc.tile_pool(name="p_mask", bufs=1)          # left
            p_mask = cm_mask.__enter__()
            mask_bf = [p_mask.tile([P, N], bf16, tag=f"mask{m}",
                                   name=f"mask{m}") for m in range(MC)]

            # ===== phase A+B fused: sq, g, negd, top-k, mask, transpose,
            # per-m AllGather chunk (fires as soon as mask m is transposed,
            # overlapping the rest of phase A). zt is streamed in 1 MB slabs
            # to keep SBUF under budget with mt_sb resident.
            cm_mt = tc.tile_pool(name="p_mt", bufs=1, side="right")
            p_mt = cm_mt.__enter__()
            mt_sb = p_mt.tile([P, TC, B], bf16, tag="mt")            # 32 KB/p
            # Gathered mask layout: per-cb tensors [r][N][P].
            ag1_in = dram.tile([MC * N, P], bf16)
            ag1_outs = [dram.tile([NC_CORES * N, P], bf16,
                                  addr_space="Shared", name=f"ag1o{cb}")
                        for cb in range(MC)]
            zt_v = zt_in.rearrange("(kc p) n -> p kc n", p=P)
            with tc.tile_pool(name="p_a", bufs=1) as p_a:
                zt_own_sb = p_a.tile([P, KC, B], f32, tag="zt_own")
                nc.sync.dma_start(
                    zt_own_sb[:], zt_own_in.rearrange("(kc p) n -> p kc n", p=P))
                sq_bcast = p_a.tile([P, N], f32, tag="sq_bcast")     # 16 KB/p

                # own-rows sq -> per-partition [P,1]; bitwise-equal to
                # sq_bcast values (same systolic + psum accumulation order)
                sq_own_cols = []
                with tc.tile_pool(name="p_zsq", bufs=1) as p_zsq:
                    zsq_own = p_zsq.tile([P, KC, B], f32, tag="zsq_own")
                    for kc in range(KC):
                        nc.vector.tensor_mul(
                            zsq_own[:, kc, :], zt_own_sb[:, kc, :],
                            zt_own_sb[:, kc, :])
                    for m in range(MC):
                        ps = ps_mm.tile([P, P], f32, tag="mm")
                        for kc in range(KC):
                            nc.tensor.matmul(
                                ps[:], ones128[:],
                                zsq_own[:, kc, m * P:(m + 1) * P],
                                start=(kc == 0), stop=(kc == KC - 1))
                        sq_bc_m = p_zsq.tile([P, P], f32, tag="sq_bc_m",
                                             bufs=2)
                        nc.scalar.copy(sq_bc_m[:], ps[:])
                        psT = ps_tp.tile([P, P], f32, tag="tp")
                        nc.tensor.transpose(psT[:], sq_bc_m[:], id32[:])
                        sqc = p_small.tile([P, 1], f32, tag=f"sq_own{m}",
                                           name=f"sq_own{m}")
                        nc.vector.tensor_copy(sqc[:], psT[:, 0:1])
                        sq_own_cols.append(sqc)

                # lhsT for g: 2 * zt_own (exact)
                zt_own2 = p_a.tile([P, KC, B], f32, tag="zt_own2")
                for kc in range(KC):
                    nc.vector.tensor_scalar_mul(
                        zt_own2[:, kc, :], zt_own_sb[:, kc, :], 2.0)

                with tc.tile_pool(name="p_tk", bufs=1) as p_tk:
                    for m in range(MC):
                        negd = p_tk.tile([P, N], f32, tag="negd", bufs=2)
                        for n in range(NCH):
                            slab = p_tk.tile([P, KC, 512], f32, tag="slab",
                                             bufs=3)
                            nc.sync.dma_start(
                                slab[:], zt_v[:, :, n * 512:(n + 1) * 512])
                            if m == 0:
                                # sq_bcast[n] (exact, fixed order)
                                zq = p_tk.tile([P, KC, 512], f32, tag="zq",
                                               bufs=2)
                                for kc in range(KC):
                                    nc.vector.tensor_mul(
                                        zq[:, kc, :], slab[:, kc, :],
                                        slab[:, kc, :])
                                psq = ps_mm.tile([P, 512], f32, tag="mm")
                                for kc in range(KC):
                                    nc.tensor.matmul(
                                        psq[:], ones128[:], zq[:, kc, :],
                                        start=(kc == 0), stop=(kc == KC - 1))
                                nc.scalar.copy(
                                    sq_bcast[:, n * 512:(n + 1) * 512],
                                    psq[:])
                            ps = ps_mm.tile([P, 512], f32, tag="mm")
                            for kc in range(KC):
                                nc.tensor.matmul(
                                    ps[:],
                                    zt_own2[:, kc, m * P:(m + 1) * P],
                                    slab[:, kc, :],
                                    start=(kc == 0), stop=(kc == KC - 1))
                            s_chunk = p_tk.tile([P, 512], f32, tag="s_chunk",
                                                bufs=2)
                            nc.vector.tensor_scalar(
                                out=s_chunk[:],
                                in0=sq_bcast[:, n * 512:(n + 1) * 512],
                                scalar1=sq_own_cols[m][:], scalar2=None,
                                op0=AL.add)
                            nc.vector.tensor_tensor(
                                out=negd[:, n * 512:(n + 1) * 512], in0=ps[:],
                                in1=s_chunk[:], op=AL.subtract)

                        mx1 = p_tk.tile([P, 8], f32, tag="mx1", bufs=2)
                        nc.vector.max(mx1[:], negd[:])
                        msk = p_tk.tile([P, N], f32, tag="msk", bufs=1)
                        nc.vector.match_replace(msk[:], mx1[:], negd[:],
                                                NEG_BIG)
                        mx2 = p_tk.tile([P, 8], f32, tag="mx2", bufs=2)
                        nc.vector.max(mx2[:], msk[:])
                        kth = p_tk.tile([P, 1], f32, tag="kth", bufs=2)
                        nc.vector.tensor_copy(kth[:], mx2[:, 1:2])

                        nc.vector.tensor_scalar(
                            out=mask_bf[m][:], in0=negd[:], scalar1=kth[:],
                            scalar2=None, op0=AL.is_ge)
                        nc.vector.reduce_sum(rs1[m][:], mask_bf[m][:],
                                             axis=AX)

                        # transpose mask m, store its AG chunk, fire AllGather
                        for a4 in range(TC // 4):
                            psT = ps_tp.tile([P, 512], bf16, tag="tp")
                            for j in range(4):
                                a = a4 * 4 + j
                                nc.tensor.transpose(
                                    psT[:, j * P:(j + 1) * P],
                                    mask_bf[m][:, a * P:(a + 1) * P], id16[:])
                            nc.scalar.copy(
                                mt_sb[:, a4 * 4:(a4 + 1) * 4,
                                      m * P:(m + 1) * P],
                                psT.rearrange("p (j q) -> p j q", j=4))
                        nc.sync.dma_start(
                            ag1_in[m * N:(m + 1) * N, :].rearrange(
                                "(a p) q -> p a q", p=P),
                            mt_sb[:, :, m * P:(m + 1) * P])
                        nc.gpsimd.collective_compute(
                            "AllGather", AL.bypass,
                            replica_groups=[list(range(NC_CORES))],
                            ins=[ag1_in[m * N:(m + 1) * N, :].opt()],
                            outs=[ag1_outs[m][:].opt()],
                        )

            # ============ phase C: R rows, AllGather #2 ============
            # X[q, j] = M[j, own q] = Mt[own row, j]; Mt row (pid*B+m*128+q),
            # col j=(r*B+c): ag1_out[(r*MC + c//128)*N + row, c%128]
            # Gathered R layout: [m][r][jb][128][512]; one AllGather chunk
            # per m, launched right after R(m) -- overlaps with R(m+1)/Ct.
            # Gathered R layout: per-m tensors [r][jb][128][512]; one
            # AllGather chunk per m, launched right after R(m).
            ag2_in = dram.tile([MC * NCH * P, 512], bf16)
            ag2_outs = [dram.tile([NC_CORES * NCH * P, 512], bf16,
                                  addr_space="Shared", name=f"ag2o{m}")
                        for m in range(MC)]
            with tc.tile_pool(name="p_r", bufs=1) as p_r:
                for m in range(MC):
                    rbf = p_r.tile([P, N], bf16, tag="rbf", bufs=2)
                    # X for all sources r, own rows m: one DMA per cb
                    xt = p_r.tile([P, MC, NC_CORES, P], bf16, tag="xt",
                                  bufs=2)
                    for cb in range(MC):
                        nc.sync.dma_start(
                            xt[:, cb, :, :],
                            ag1_outs[cb].rearrange(
                                "(r a) q -> a r q", r=NC_CORES)[
                                bass.ds(pid * B + m * P, P), :, :])
                    for r in range(NC_CORES):
                        nc.vector.tensor_tensor(
                            out=rbf[:, r * B:(r + 1) * B].rearrange(
                                "p (c q) -> p c q", c=MC),
                            in0=mask_bf[m][:, r * B:(r + 1) * B].rearrange(
                                "p (c q) -> p c q", c=MC),
                            in1=xt[:, :, r, :],
                            op=AL.mult)
                    nc.vector.reduce_sum(rsR[m][:], rbf[:], axis=AX)
                    nc.sync.dma_start(
                        ag2_in[m * NCH * P:(m + 1) * NCH * P, :].rearrange(
                            "(jb q) j -> q jb j", jb=NCH),
                        rbf.rearrange("p (jb j) -> p jb j", jb=NCH))
                    nc.gpsimd.collective_compute(
                        "AllGather", AL.bypass,
                        replica_groups=[list(range(NC_CORES))],
                        ins=[ag2_in[m * NCH * P:(m + 1) * NCH * P, :].opt()],
                        outs=[ag2_outs[m][:].opt()],
                    )
            cm_mask.__exit__(None, None, None)   # masks dead

            # scale = 0.5 / (rs1 * rsR)
            halfrec = []
            for m in range(MC):
                prod = p_small.tile([P, 1], f32, tag=f"prod{m}",
                                    name=f"prod{m}")
                nc.vector.tensor_tensor(out=prod[:], in0=rs1[m][:],
                                        in1=rsR[m][:], op=AL.mult)
                rec = p_small.tile([P, 1], f32, tag=f"rec{m}",
                                   name=f"rec{m}")
                nc.vector.reciprocal(rec[:], prod[:])
                hr = p_small.tile([P, 1], f32, tag=f"hr{m}", name=f"hr{m}")
                nc.vector.tensor_scalar_mul(hr[:], rec[:], 0.5)
                halfrec.append(hr)

            # ============ phase D: Ct = (M M^T)[:, own] ============
            cm_ct = tc.tile_pool(name="p_ct", bufs=1)              # left
            p_ct = cm_ct.__enter__()
            ct_sb = p_ct.tile([P, TC, B], bf16, tag="ct")            # 32 KB/p
            with tc.tile_pool(name="p_lh", bufs=1) as p_lh:
                for t in range(TC):
                    # t covers Mt cols [t*128, +128): source core r = t//MC,
                    # sub-block cb = t%MC -> gathered block (cb, r).
                    lh = p_lh.tile([P, TC, P], bf16, tag="lh", bufs=3)
                    nc.scalar.dma_start(
                        lh[:],
                        ag1_outs[t % MC][(t // MC) * N:(t // MC + 1) * N, :]
                        .rearrange("(a p) q -> p a q", p=P))
                    ps = ps_mm.tile([P, B], f32, tag="mm")
                    for kc in range(TC):
                        nc.tensor.matmul(
                            ps[:], lh[:, kc, :], mt_sb[:, kc, :],
                            start=(kc == 0), stop=(kc == TC - 1))
                    nc.scalar.copy(ct_sb[:, t, :], ps[:])
            cm_mt.__exit__(None, None, None)     # mt dead

            # ============ phase E: S2 rows = scale * (Ct^T @ R) ============
            cm_s2 = tc.tile_pool(name="p_s2", bufs=1, side="right")
            p_s2 = cm_s2.__enter__()
            sim2 = [p_s2.tile([P, N], f32, tag=f"sim2_{m}", name=f"sim2_{m}")
                    for m in range(MC)]
            tkc = B // P
            with tc.tile_pool(name="p_w", bufs=1) as p_w:
                for n in range(NCH):
                    rsl = p_w.tile([P, NC_CORES, tkc, 512], bf16,
                                   tag="rsl", bufs=2)
                    for m in range(tkc):
                        nc.sync.dma_start(
                            rsl[:, :, m, :],
                            ag2_outs[m].rearrange(
                                "(r jb q) j -> r jb q j", r=NC_CORES,
                                jb=NCH)[:, n, :, :]
                            .rearrange("r q j -> q r j"))
                    for m in range(MC):
                        ps = ps_mm.tile([P, 512], f32, tag="mm")
                        for t in range(TC):
                            nc.tensor.matmul(
                                ps[:], ct_sb[:, t, m * P:(m + 1) * P],
                                rsl[:, t // tkc, t % tkc, :],
                                start=(t == 0), stop=(t == TC - 1))
                        nc.scalar.activation(
                            sim2[m][:, n * 512:(n + 1) * 512], ps[:],
                            mybir.ActivationFunctionType.Copy,
                            scale=halfrec[m][:])
            cm_ct.__exit__(None, None, None)     # ct dead

            # ============ phase F: AllToAll + transpose + add ============
            a2a_in = dram.tile([N, B], f32)
            for m in range(MC):
                nc.sync.dma_start(
                    a2a_in.rearrange("(r i) j -> i r j", r=NC_CORES)[
                        m * P:(m + 1) * P, :, :],
                    sim2[m].rearrange("p (r j) -> p r j", r=NC_CORES))
            a2a_out = dram.tile([N, B], f32)
            nc.gpsimd.collective_compute(
                "AllToAll", AL.bypass,
                replica_groups=[list(range(NC_CORES))],
                ins=[a2a_in[:].opt()], outs=[a2a_out[:].opt()],
            )

            with tc.tile_pool(name="p_f", bufs=1) as p_f:
                for a4 in range(TC // 4):
                    recvs = []
                    for j in range(4):
                        a = a4 * 4 + j
                        rv = p_f.tile([P, B], f32, tag="rv", bufs=8)
                        nc.sync.dma_start(rv[:], a2a_out[a * P:(a + 1) * P, :])
                        recvs.append(rv)
                    for m in range(MC):
                        ps = ps_tp.tile([P, 512], f32, tag="ps_f", bufs=2)
                        for j in range(4):
                            nc.tensor.transpose(
                                ps[:, j * P:(j + 1) * P],
                                recvs[j][:, m * P:(m + 1) * P], id32[:])
                        ob = p_f.tile([P, 512], f32, tag="ob", bufs=3)
                        nc.vector.tensor_tensor(
                            out=ob[:], in0=ps[:],
                            in1=sim2[m][:, a4 * 512:(a4 + 1) * 512],
                            op=AL.add)
                        nc.sync.dma_start(
                            out_ext[m * P:(m + 1) * P,
                                    a4 * 512:(a4 + 1) * 512],
                            ob[:])
            cm_s2.__exit__(None, None, None)

    nc.compile()
    return nc


def _get_nc():
    if "nc" not in _cache:
        _cache["nc"] = _build()
    return _cache["nc"]


def _bf16(x):
    try:
        import ml_dtypes
        return x.astype(ml_dtypes.bfloat16)
    except ImportError:
        from concourse import mybir
        return x.astype(mybir.dt.np(mybir.dt.bfloat16))


def kernel(z, _profile=False):
    from concourse import bass_utils

    z = np.ascontiguousarray(np.asarray(z, dtype=np.float32))
    assert z.shape == (N, D), z.shape
    zT = np.ascontiguousarray(z.T)
    id32 = np.eye(P, dtype=np.float32)
    id16 = _bf16(np.eye(P, dtype=np.float32))

    nc = _get_nc()
    in_maps = []
    for c in range(NC_CORES):
        in_maps.append({
            "zt": zT,
            "zt_own": np.ascontiguousarray(zT[:, c * B:(c + 1) * B]),
            "id32": id32,
            "id16": id16,
        })
    res = bass_utils.run_bass_kernel_spmd(
        nc, in_maps, core_ids=list(range(NC_CORES)), trace=_profile)
    out = np.concatenate(
        [res.results[c]["out"] for c in range(NC_CORES)], axis=0)
    if _profile:
        return out, res
    return out


# revision 21
# speedup vs baseline: 1.2720x; 1.1543x over previous
"""Contextual kNN similarity kernel for Trainium2, 8 NeuronCores.

For z [4096, 512] fp32 computes (matching reference.py's structure):
    d   = sq_i + sq_j - 2 z z^T
    kth = 10th smallest d per row (ties included via <=)
    M   = (d <= kth)                       [N, N] 0/1
    S1  = (M M^T) / rowsum(M)
    R   = M * M^T
    S2  = (S1 @ R^T) / rowsum(R)           (R symmetric)
    out = 0.5 (S2 + S2^T)

Distribution: row-shard over 8 cores (512 rows each, z replicated).
Each core: row block of -d (fp32 PE matmul mirroring the reference's
rounding), local top-10 via DVE max8/match_replace/max8, 0/1 mask in
bf16 (exact). PE-transpose mask -> AllGather (bf16). R rows = M .*
gathered-columns (exact), second AllGather. Then two big matmuls with
exact small-integer arithmetic (bf16 inputs, fp32 PSUM):
  Ct = (M M^T)[:, own]   and   W = sum_t C[t, i] R[t, j]
Both row-normalizations and the final 0.5 fold into one per-row scale
0.5/(rs1*rsR) applied on PSUM evacuation. Symmetrization: AllToAll of
the scaled S2, PE transpose, add. Host only preps/replicates inputs and
concatenates the 8 row-block outputs.
"""
import numpy as np

NC_CORES = 8
N = 4096
D = 512
B = N // NC_CORES        # 512 rows per core
P = 128
MC = B // P              # 4 m-chunks of own rows
NCH = N // 512           # 8 n-chunks
KC = D // P              # 4 contraction chunks for g
TC = N // P              # 32 t/k chunks
NEG_BIG = -3.0e38

_cache = {}


def _build():
    from concourse import bass, bacc, tile, mybir

    f32 = mybir.dt.float32
    bf16 = mybir.dt.bfloat16
    fp8 = mybir.dt.float8e4
    fp16 = mybir.dt.float16
    AL = mybir.AluOpType
    AX = mybir.AxisListType.X

    nc = bacc.Bacc(
        "TRN2",
        target_bir_lowering=False,
        debug=False,
        enable_asserts=False,
        num_devices=NC_CORES,
    )

    zt_in = nc.dram_tensor("zt", [D, N], f32, kind="ExternalInput").ap()
    zt_own_in = nc.dram_tensor("zt_own", [D, B], f32, kind="ExternalInput").ap()
    id32_in = nc.dram_tensor("id32", [P, P], f32, kind="ExternalInput").ap()
    id16_in = nc.dram_tensor("id16", [P, P], bf16, kind="ExternalInput").ap()
    id8_in = nc.dram_tensor("id8", [P, P], fp8, kind="ExternalInput").ap()
    idh_in = nc.dram_tensor("idh", [P, P], fp16, kind="ExternalInput").ap()
    out_ext = nc.dram_tensor("out", [B, N], f32, kind="ExternalOutput").ap()

    with tile.TileContext(nc) as tc:
        with tc.tile_pool(name="ps_mm", bufs=3, space="PSUM") as ps_mm, \
             tc.tile_pool(name="ps_tp", bufs=2, space="PSUM") as ps_tp, \
             tc.tile_pool(name="dram", bufs=1, space="DRAM") as dram, \
             tc.tile_pool(name="p_small", bufs=1) as p_small:

            pid = nc.sync.partition_id()

            ones128 = p_small.tile([P, P], f32, tag="ones128")
            nc.vector.memset(ones128[:], 1.0)
            id32 = p_small.tile([P, P], f32, tag="id32")
            nc.sync.dma_start(id32[:], id32_in[:])
            id16 = p_small.tile([P, P], bf16, tag="id16")
            nc.sync.dma_start(id16[:], id16_in[:])
            id8 = p_small.tile([P, P], fp8, tag="id8")
            nc.sync.dma_start(id8[:], id8_in[:])
            idh = p_small.tile([P, P], fp16, tag="idh")
            nc.sync.dma_start(idh[:], idh_in[:])

            rs1 = [p_small.tile([P, 1], f32, tag=f"rs1_{m}", name=f"rs1_{m}")
                   for m in range(MC)]
            rsR = [p_small.tile([P, 1], f32, tag=f"rsR{m}", name=f"rsR{m}")
                   for m in range(MC)]

            # Overlapping lifetimes (mask A..C, mt B..D, ct D..E, s2 E..F):
            # pools must release in per-side LIFO order, so alternate the
            # chain between the left and right SBUF stacks.
            cm_mask = tc.tile_pool(name="p_mask", bufs=1)          # left
            p_mask = cm_mask.__enter__()
            mask_bf = [p_mask.tile([P, N], fp8, tag=f"mask{m}",
                                   name=f"mask{m}") for m in range(MC)]

            # ===== phase A+B fused: sq, g, negd, top-k, mask, transpose,
            # per-m AllGather chunk (fires as soon as mask m is transposed,
            # overlapping the rest of phase A). zt is streamed in 1 MB slabs
            # to keep SBUF under budget with mt_sb resident.
            cm_mt = tc.tile_pool(name="p_mt", bufs=1, side="right")
            p_mt = cm_mt.__enter__()
            mt_sb = p_mt.tile([P, TC, B], fp8, tag="mt")            # 32 KB/p
            # Gathered mask layout: per-cb tensors [r][N][P].
            ag1_in = dram.tile([MC * N, P], fp8)
            ag1_outs = [dram.tile([NC_CORES * N, P], fp8,
                                  addr_space="Shared", name=f"ag1o{cb}")
                        for cb in range(MC)]
            zt_v = zt_in.rearrange("(kc p) n -> p kc n", p=P)
            with tc.tile_pool(name="p_a", bufs=1) as p_a:
                zt_own_sb = p_a.tile([P, KC, B], f32, tag="zt_own")
                nc.sync.dma_start(
                    zt_own_sb[:], zt_own_in.rearrange("(kc p) n -> p kc n", p=P))
                sq_bcast = p_a.tile([P, N], f32, tag="sq_bcast")     # 16 KB/p

                # own-rows sq -> per-partition [P,1]; bitwise-equal to
                # sq_bcast values (same systolic + psum accumulation order)
                sq_own_cols = []
                with tc.tile_pool(name="p_zsq", bufs=1) as p_zsq:
                    zsq_own = p_zsq.tile([P, KC, B], f32, tag="zsq_own")
                    for kc in range(KC):
                        nc.vector.tensor_mul(
                            zsq_own[:, kc, :], zt_own_sb[:, kc, :],
                            zt_own_sb[:, kc, :])
                    for m in range(MC):
                        ps = ps_mm.tile([P, P], f32, tag="mm")
                        for kc in range(KC):
                            nc.tensor.matmul(
                                ps[:], ones128[:],
                                zsq_own[:, kc, m * P:(m + 1) * P],
                                start=(kc == 0), stop=(kc == KC - 1))
                        sq_bc_m = p_zsq.tile([P, P], f32, tag="sq_bc_m",
                                             bufs=2)
                        nc.scalar.copy(sq_bc_m[:], ps[:])
                        psT = ps_tp.tile([P, P], f32, tag="tp")
                        nc.tensor.transpose(psT[:], sq_bc_m[:], id32[:])
                        sqc = p_small.tile([P, 1], f32, tag=f"sq_own{m}",
                                           name=f"sq_own{m}")
                        nc.vector.tensor_copy(sqc[:], psT[:, 0:1])
                        sq_own_cols.append(sqc)

                # lhsT for g: 2 * zt_own (exact)
                zt_own2 = p_a.tile([P, KC, B], f32, tag="zt_own2")
                for kc in range(KC):
                    nc.vector.tensor_scalar_mul(
                        zt_own2[:, kc, :], zt_own_sb[:, kc, :], 2.0)

                with tc.tile_pool(name="p_tk", bufs=1) as p_tk:
                    for m in range(MC):
                        negd = p_tk.tile([P, N], f32, tag="negd", bufs=2)
                        for n in range(NCH):
                            slab = p_tk.tile([P, KC, 512], f32, tag="slab",
                                             bufs=3)
                            nc.sync.dma_start(
                                slab[:], zt_v[:, :, n * 512:(n + 1) * 512])
                            if m == 0:
                                # sq_bcast[n] (exact, fixed order)
                                zq = p_tk.tile([P, KC, 512], f32, tag="zq",
                                               bufs=2)
                                for kc in range(KC):
                                    nc.vector.tensor_mul(
                                        zq[:, kc, :], slab[:, kc, :],
                                        slab[:, kc, :])
                                psq = ps_mm.tile([P, 512], f32, tag="mm")
                                for kc in range(KC):
                                    nc.tensor.matmul(
                                        psq[:], ones128[:], zq[:, kc, :],
                                        start=(kc == 0), stop=(kc == KC - 1))
                                nc.scalar.copy(
                                    sq_bcast[:, n * 512:(n + 1) * 512],
                                    psq[:])
                            ps = ps_mm.tile([P, 512], f32, tag="mm")
                            for kc in range(KC):
                                nc.tensor.matmul(
                                    ps[:],
                                    zt_own2[:, kc, m * P:(m + 1) * P],
                                    slab[:, kc, :],
                                    start=(kc == 0), stop=(kc == KC - 1))
                            s_chunk = p_tk.tile([P, 512], f32, tag="s_chunk",
                                                bufs=2)
                            nc.vector.tensor_scalar(
                                out=s_chunk[:],
                                in0=sq_bcast[:, n * 512:(n + 1) * 512],
                                scalar1=sq_own_cols[m][:], scalar2=None,
                                op0=AL.add)
                            nc.vector.tensor_tensor(
                                out=negd[:, n * 512:(n + 1) * 512], in0=ps[:],
                                in1=s_chunk[:], op=AL.subtract)

                        mx1 = p_tk.tile([P, 8], f32, tag="mx1", bufs=2)
                        nc.vector.max(mx1[:], negd[:])
                        msk = p_tk.tile([P, N], f32, tag="msk", bufs=1)
                        nc.vector.match_replace(msk[:], mx1[:], negd[:],
                                                NEG_BIG)
                        mx2 = p_tk.tile([P, 8], f32, tag="mx2", bufs=2)
                        nc.vector.max(mx2[:], msk[:])
                        kth = p_tk.tile([P, 1], f32, tag="kth", bufs=2)
                        nc.vector.tensor_copy(kth[:], mx2[:, 1:2])

                        nc.vector.tensor_scalar(
                            out=mask_bf[m][:], in0=negd[:], scalar1=kth[:],
                            scalar2=None, op0=AL.is_ge)
                        nc.vector.reduce_sum(rs1[m][:], mask_bf[m][:],
                                             axis=AX)

                        # transpose mask m, store its AG chunk, fire AllGather
                        for a4 in range(TC // 4):
                            # fp8 transpose-mode requires output element
                            # step 2 in PSUM; evacuate with the same stride.
                            psT = ps_tp.tile([P, 4, P, 2], fp8, tag="tp")
                            for j in range(4):
                                a = a4 * 4 + j
                                nc.tensor.transpose(
                                    psT[:, j, :, 0],
                                    mask_bf[m][:, a * P:(a + 1) * P], id8[:])
                            nc.scalar.copy(
                                mt_sb[:, a4 * 4:(a4 + 1) * 4,
                                      m * P:(m + 1) * P],
                                psT[:, :, :, 0])
                        nc.sync.dma_start(
                            ag1_in[m * N:(m + 1) * N, :].rearrange(
                                "(a p) q -> p a q", p=P),
                            mt_sb[:, :, m * P:(m + 1) * P])
                        nc.gpsimd.collective_compute(
                            "AllGather", AL.bypass,
                            replica_groups=[list(range(NC_CORES))],
                            ins=[ag1_in[m * N:(m + 1) * N, :].opt()],
                            outs=[ag1_outs[m][:].opt()],
                        )

            # ============ phase C: R rows, AllGather #2 ============
            # X[q, j] = M[j, own q] = Mt[own row, j]; Mt row (pid*B+m*128+q),
            # col j=(r*B+c): ag1_out[(r*MC + c//128)*N + row, c%128]
            # Gathered R layout: [m][r][jb][128][512]; one AllGather chunk
            # per m, launched right after R(m) -- overlaps with R(m+1)/Ct.
            # Gathered R layout: per-m tensors [r][jb][128][512]; one
            # AllGather chunk per m, launched right after R(m).
            ag2_in = dram.tile([MC * NCH * P, 512], fp8)
            ag2_outs = [dram.tile([NC_CORES * NCH * P, 512], fp8,
                                  addr_space="Shared", name=f"ag2o{m}")
                        for m in range(MC)]
            with tc.tile_pool(name="p_r", bufs=1) as p_r:
                for m in range(MC):
                    rbf = p_r.tile([P, N], fp8, tag="rbf", bufs=2)
                    # X for all sources r, own rows m: one DMA per cb
                    xt = p_r.tile([P, MC, NC_CORES, P], fp8, tag="xt",
                                  bufs=2)
                    for cb in range(MC):
                        nc.sync.dma_start(
                            xt[:, cb, :, :],
                            ag1_outs[cb].rearrange(
                                "(r a) q -> a r q", r=NC_CORES)[
                                bass.ds(pid * B + m * P, P), :, :])
                    for r in range(NC_CORES):
                        nc.vector.tensor_tensor(
                            out=rbf[:, r * B:(r + 1) * B].rearrange(
                                "p (c q) -> p c q", c=MC),
                            in0=mask_bf[m][:, r * B:(r + 1) * B].rearrange(
                                "p (c q) -> p c q", c=MC),
                            in1=xt[:, :, r, :],
                            op=AL.mult)
                    nc.vector.reduce_sum(rsR[m][:], rbf[:], axis=AX)
                    nc.sync.dma_start(
                        ag2_in[m * NCH * P:(m + 1) * NCH * P, :].rearrange(
                            "(jb q) j -> q jb j", jb=NCH),
                        rbf.rearrange("p (jb j) -> p jb j", jb=NCH))
                    nc.gpsimd.collective_compute(
                        "AllGather", AL.bypass,
                        replica_groups=[list(range(NC_CORES))],
                        ins=[ag2_in[m * NCH * P:(m + 1) * NCH * P, :].opt()],
                        outs=[ag2_outs[m][:].opt()],
                    )
            cm_mask.__exit__(None, None, None)   # masks dead

            # scale = 0.5 / (rs1 * rsR)
            halfrec = []
            for m in range(MC):
                prod = p_small.tile([P, 1], f32, tag=f"prod{m}",
                                    name=f"prod{m}")
                nc.vector.tensor_tensor(out=prod[:], in0=rs1[m][:],
                                        in1=rsR[m][:], op=AL.mult)
                rec = p_small.tile([P, 1], f32, tag=f"rec{m}",
                                   name=f"rec{m}")
                nc.vector.reciprocal(rec[:], prod[:])
                hr = p_small.tile([P, 1], f32, tag=f"hr{m}", name=f"hr{m}")
                nc.vector.tensor_scalar_mul(hr[:], rec[:], 0.5)
                halfrec.append(hr)

            # ============ phase D: Ct = (M M^T)[:, own] ============
            cm_ct = tc.tile_pool(name="p_ct", bufs=1)              # left
            p_ct = cm_ct.__enter__()
            ct_sb = p_ct.tile([P, TC, B], fp8, tag="ct")            # 32 KB/p
            with tc.tile_pool(name="p_lh", bufs=1) as p_lh:
                # consume AG1 chunks in arrival order (cb outer); two source
                # cores per weight-slab DMA (2 MB contiguous).
                for cb in range(MC):
                    for r2 in range(NC_CORES // 2):
                        lh = p_lh.tile([P, 2, TC, P], fp8, tag="lh", bufs=3)
                        nc.scalar.dma_start(
                            lh[:],
                            ag1_outs[cb][r2 * 2 * N:(r2 * 2 + 2) * N, :]
                            .rearrange("(r a p) q -> p r a q", r=2, p=P))
                        for h in range(2):
                            t = (r2 * 2 + h) * MC + cb
                            ps = ps_mm.tile([P, B], f32, tag="mm")
                            for kc in range(TC):
                                nc.tensor.matmul(
                                    ps[:], lh[:, h, kc, :], mt_sb[:, kc, :],
                                    start=(kc == 0), stop=(kc == TC - 1))
                            nc.scalar.copy(ct_sb[:, t, :], ps[:])
            cm_mt.__exit__(None, None, None)     # mt dead

            # ============ phase E: S2 rows = scale * (Ct^T @ R) ============
            cm_s2 = tc.tile_pool(name="p_s2", bufs=1, side="right")
            p_s2 = cm_s2.__enter__()
            sim2 = [p_s2.tile([P, N], f32, tag=f"sim2_{m}", name=f"sim2_{m}")
                    for m in range(MC)]
            tkc = B // P
            with tc.tile_pool(name="p_w", bufs=1) as p_w:
                for n in range(NCH):
                    rsl = p_w.tile([P, NC_CORES, tkc, 512], fp8,
                                   tag="rsl", bufs=2)
                    for m in range(tkc):
                        nc.sync.dma_start(
                            rsl[:, :, m, :],
                            ag2_outs[m].rearrange(
                                "(r jb q) j -> r jb q j", r=NC_CORES,
                                jb=NCH)[:, n, :, :]
                            .rearrange("r q j -> q r j"))
                    for m in range(MC):
                        ps = ps_mm.tile([P, 512], f32, tag="mm")
                        for t in range(TC):
                            nc.tensor.matmul(
                                ps[:], ct_sb[:, t, m * P:(m + 1) * P],
                                rsl[:, t // tkc, t % tkc, :],
                                start=(t == 0), stop=(t == TC - 1))
                        nc.scalar.activation(
                            sim2[m][:, n * 512:(n + 1) * 512], ps[:],
                            mybir.ActivationFunctionType.Copy,
                            scale=halfrec[m][:])
            cm_ct.__exit__(None, None, None)     # ct dead

            # ============ phase F: AllToAll + transpose + add ============
            # Payload in fp16: the transposed half of the symmetrization
            # carries ~5e-4 relative error on values <= 1, far inside the
            # tolerance, and halves the AllToAll bytes.
            a2a_in = dram.tile([N, B], fp16)
            for m in range(MC):
                s16 = p_s2.tile([P, N], fp16, tag="s16", bufs=2)
                nc.vector.tensor_copy(s16[:], sim2[m][:])
                nc.sync.dma_start(
                    a2a_in.rearrange("(r i) j -> i r j", r=NC_CORES)[
                        m * P:(m + 1) * P, :, :],
                    s16.rearrange("p (r j) -> p r j", r=NC_CORES))
            a2a_out = dram.tile([N, B], fp16)
            nc.gpsimd.collective_compute(
                "AllToAll", AL.bypass,
                replica_groups=[list(range(NC_CORES))],
                ins=[a2a_in[:].opt()], outs=[a2a_out[:].opt()],
            )

            with tc.tile_pool(name="p_f", bufs=1) as p_f:
                for a4 in range(TC // 4):
                    recvs = []
                    for j in range(4):
                        a = a4 * 4 + j
                        rv = p_f.tile([P, B], fp16, tag="rv", bufs=8)
                        nc.sync.dma_start(rv[:], a2a_out[a * P:(a + 1) * P, :])
                        recvs.append(rv)
                    for m in range(MC):
                        ps = ps_tp.tile([P, 512], fp16, tag="ps_f", bufs=2)
                        for j in range(4):
                            nc.tensor.transpose(
                                ps[:, j * P:(j + 1) * P],
                                recvs[j][:, m * P:(m + 1) * P], idh[:])
                        ob = p_f.tile([P, 512], f32, tag="ob", bufs=3)
                        nc.vector.tensor_tensor(
                            out=ob[:], in0=ps[:],
                            in1=sim2[m][:, a4 * 512:(a4 + 1) * 512],
                            op=AL.add)
                        nc.sync.dma_start(
                            out_ext[m * P:(m + 1) * P,
                                    a4 * 512:(a4 + 1) * 512],
                            ob[:])
            cm_s2.__exit__(None, None, None)

    nc.compile()
    return nc


def _get_nc():
    if "nc" not in _cache:
        _cache["nc"] = _build()
    return _cache["nc"]


def _bf16(x):
    try:
        import ml_dtypes
        return x.astype(ml_dtypes.bfloat16)
    except ImportError:
        from concourse import mybir
        return x.astype(mybir.dt.np(mybir.dt.bfloat16))


def kernel(z, _profile=False):
    from concourse import bass_utils

    from concourse import mybir
    z = np.ascontiguousarray(np.asarray(z, dtype=np.float32))
    assert z.shape == (N, D), z.shape
    zT = np.ascontiguousarray(z.T)
    eye = np.eye(P, dtype=np.float32)
    id32 = eye
    id16 = _bf16(eye)
    id8 = eye.astype(mybir.dt.np(mybir.dt.float8e4))
    idh = eye.astype(np.float16)

    nc = _get_nc()
    in_maps = []
    for c in range(NC_CORES):
        in_maps.append({
            "zt": zT,
            "zt_own": np.ascontiguousarray(zT[:, c * B:(c + 1) * B]),
            "id32": id32,
            "id16": id16,
            "id8": id8,
            "idh": idh,
        })
    res = bass_utils.run_bass_kernel_spmd(
        nc, in_maps, core_ids=list(range(NC_CORES)), trace=_profile)
    out = np.concatenate(
        [res.results[c]["out"] for c in range(NC_CORES)], axis=0)
    if _profile:
        return out, res
    return out


# revision 22
# speedup vs baseline: 1.2731x; 1.0008x over previous
"""Contextual kNN similarity kernel for Trainium2, 8 NeuronCores.

For z [4096, 512] fp32 computes (matching reference.py's structure):
    d   = sq_i + sq_j - 2 z z^T
    kth = 10th smallest d per row (ties included via <=)
    M   = (d <= kth)                       [N, N] 0/1
    S1  = (M M^T) / rowsum(M)
    R   = M * M^T
    S2  = (S1 @ R^T) / rowsum(R)           (R symmetric)
    out = 0.5 (S2 + S2^T)

Distribution: row-shard over 8 cores (512 rows each, z replicated).
Each core: row block of -d (fp32 PE matmul mirroring the reference's
rounding), local top-10 via DVE max8/match_replace/max8, 0/1 mask in
bf16 (exact). PE-transpose mask -> AllGather (bf16). R rows = M .*
gathered-columns (exact), second AllGather. Then two big matmuls with
exact small-integer arithmetic (bf16 inputs, fp32 PSUM):
  Ct = (M M^T)[:, own]   and   W = sum_t C[t, i] R[t, j]
Both row-normalizations and the final 0.5 fold into one per-row scale
0.5/(rs1*rsR) applied on PSUM evacuation. Symmetrization: AllToAll of
the scaled S2, PE transpose, add. Host only preps/replicates inputs and
concatenates the 8 row-block outputs.
"""
import numpy as np

NC_CORES = 8
N = 4096
D = 512
B = N // NC_CORES        # 512 rows per core
P = 128
MC = B // P              # 4 m-chunks of own rows
NCH = N // 512           # 8 n-chunks
KC = D // P              # 4 contraction chunks for g
TC = N // P              # 32 t/k chunks
NEG_BIG = -3.0e38

_cache = {}


def _build():
    from concourse import bass, bacc, tile, mybir

    f32 = mybir.dt.float32
    bf16 = mybir.dt.bfloat16
    fp8 = mybir.dt.float8e4
    fp16 = mybir.dt.float16
    AL = mybir.AluOpType
    AX = mybir.AxisListType.X

    nc = bacc.Bacc(
        "TRN2",
        target_bir_lowering=False,
        debug=False,
        enable_asserts=False,
        num_devices=NC_CORES,
    )

    zt_in = nc.dram_tensor("zt", [D, N], f32, kind="ExternalInput").ap()
    zt_own_in = nc.dram_tensor("zt_own", [D, B], f32, kind="ExternalInput").ap()
    id32_in = nc.dram_tensor("id32", [P, P], f32, kind="ExternalInput").ap()
    id16_in = nc.dram_tensor("id16", [P, P], bf16, kind="ExternalInput").ap()
    id8_in = nc.dram_tensor("id8", [P, P], fp8, kind="ExternalInput").ap()
    idh_in = nc.dram_tensor("idh", [P, P], fp16, kind="ExternalInput").ap()
    out_ext = nc.dram_tensor("out", [B, N], f32, kind="ExternalOutput").ap()

    with tile.TileContext(nc) as tc:
        with tc.tile_pool(name="ps_mm", bufs=3, space="PSUM") as ps_mm, \
             tc.tile_pool(name="ps_tp", bufs=2, space="PSUM") as ps_tp, \
             tc.tile_pool(name="dram", bufs=1, space="DRAM") as dram, \
             tc.tile_pool(name="p_small", bufs=1) as p_small:

            pid = nc.sync.partition_id()

            ones128 = p_small.tile([P, P], f32, tag="ones128")
            nc.vector.memset(ones128[:], 1.0)
            id32 = p_small.tile([P, P], f32, tag="id32")
            nc.sync.dma_start(id32[:], id32_in[:])
            id16 = p_small.tile([P, P], bf16, tag="id16")
            nc.sync.dma_start(id16[:], id16_in[:])
            id8 = p_small.tile([P, P], fp8, tag="id8")
            nc.sync.dma_start(id8[:], id8_in[:])
            idh = p_small.tile([P, P], fp16, tag="idh")
            nc.sync.dma_start(idh[:], idh_in[:])

            rs1 = [p_small.tile([P, 1], f32, tag=f"rs1_{m}", name=f"rs1_{m}")
                   for m in range(MC)]
            rsR = [p_small.tile([P, 1], f32, tag=f"rsR{m}", name=f"rsR{m}")
                   for m in range(MC)]

            # Overlapping lifetimes (mask A..C, mt B..D, ct D..E, s2 E..F):
            # pools must release in per-side LIFO order, so alternate the
            # chain between the left and right SBUF stacks.
            cm_mask = tc.tile_pool(name="p_mask", bufs=1)          # left
            p_mask = cm_mask.__enter__()
            mask_bf = [p_mask.tile([P, N], fp8, tag=f"mask{m}",
                                   name=f"mask{m}") for m in range(MC)]

            # ===== phase A+B fused: sq, g, negd, top-k, mask, transpose,
            # per-m AllGather chunk (fires as soon as mask m is transposed,
            # overlapping the rest of phase A). zt is streamed in 1 MB slabs
            # to keep SBUF under budget with mt_sb resident.
            cm_mt = tc.tile_pool(name="p_mt", bufs=1, side="right")
            p_mt = cm_mt.__enter__()
            mt_sb = p_mt.tile([P, TC, B], fp8, tag="mt")            # 32 KB/p
            # Gathered mask layout: per-cb tensors [r][N][P].
            ag1_ins = [dram.tile([N, P], fp8, name=f"ag1i{cb}")
                       for cb in range(MC)]
            ag1_outs = [dram.tile([NC_CORES * N, P], fp8,
                                  addr_space="Shared", name=f"ag1o{cb}")
                        for cb in range(MC)]
            zt_v = zt_in.rearrange("(kc p) n -> p kc n", p=P)
            with tc.tile_pool(name="p_a", bufs=1) as p_a:
                zt_own_sb = p_a.tile([P, KC, B], f32, tag="zt_own")
                nc.sync.dma_start(
                    zt_own_sb[:], zt_own_in.rearrange("(kc p) n -> p kc n", p=P))
                sq_bcast = p_a.tile([P, N], f32, tag="sq_bcast")     # 16 KB/p

                # own-rows sq -> per-partition [P,1]; bitwise-equal to
                # sq_bcast values (same systolic + psum accumulation order)
                sq_own_cols = []
                with tc.tile_pool(name="p_zsq", bufs=1) as p_zsq:
                    zsq_own = p_zsq.tile([P, KC, B], f32, tag="zsq_own")
                    for kc in range(KC):
                        nc.vector.tensor_mul(
                            zsq_own[:, kc, :], zt_own_sb[:, kc, :],
                            zt_own_sb[:, kc, :])
                    for m in range(MC):
                        ps = ps_mm.tile([P, P], f32, tag="mm")
                        for kc in range(KC):
                            nc.tensor.matmul(
                                ps[:], ones128[:],
                                zsq_own[:, kc, m * P:(m + 1) * P],
                                start=(kc == 0), stop=(kc == KC - 1))
                        sq_bc_m = p_zsq.tile([P, P], f32, tag="sq_bc_m",
                                             bufs=2)
                        nc.scalar.copy(sq_bc_m[:], ps[:])
                        psT = ps_tp.tile([P, P], f32, tag="tp")
                        nc.tensor.transpose(psT[:], sq_bc_m[:], id32[:])
                        sqc = p_small.tile([P, 1], f32, tag=f"sq_own{m}",
                                           name=f"sq_own{m}")
                        nc.vector.tensor_copy(sqc[:], psT[:, 0:1])
                        sq_own_cols.append(sqc)

                # lhsT for g: 2 * zt_own (exact)
                zt_own2 = p_a.tile([P, KC, B], f32, tag="zt_own2")
                for kc in range(KC):
                    nc.vector.tensor_scalar_mul(
                        zt_own2[:, kc, :], zt_own_sb[:, kc, :], 2.0)

                with tc.tile_pool(name="p_tk", bufs=1) as p_tk:
                    for m in range(MC):
                        negd = p_tk.tile([P, N], f32, tag="negd", bufs=2)
                        for n in range(NCH):
                            slab = p_tk.tile([P, KC, 512], f32, tag="slab",
                                             bufs=3)
                            nc.sync.dma_start(
                                slab[:], zt_v[:, :, n * 512:(n + 1) * 512])
                            if m == 0:
                                # sq_bcast[n] (exact, fixed order)
                                zq = p_tk.tile([P, KC, 512], f32, tag="zq",
                                               bufs=2)
                                for kc in range(KC):
                                    nc.vector.tensor_mul(
                                        zq[:, kc, :], slab[:, kc, :],
                                        slab[:, kc, :])
                                psq = ps_mm.tile([P, 512], f32, tag="mm")
                                for kc in range(KC):
                                    nc.tensor.matmul(
                                        psq[:], ones128[:], zq[:, kc, :],
                                        start=(kc == 0), stop=(kc == KC - 1))
                                nc.scalar.copy(
                                    sq_bcast[:, n * 512:(n + 1) * 512],
                                    psq[:])
                            ps = ps_mm.tile([P, 512], f32, tag="mm")
                            for kc in range(KC):
                                nc.tensor.matmul(
                                    ps[:],
                                    zt_own2[:, kc, m * P:(m + 1) * P],
                                    slab[:, kc, :],
                                    start=(kc == 0), stop=(kc == KC - 1))
                            s_chunk = p_tk.tile([P, 512], f32, tag="s_chunk",
                                                bufs=2)
                            nc.vector.tensor_scalar(
                                out=s_chunk[:],
                                in0=sq_bcast[:, n * 512:(n + 1) * 512],
                                scalar1=sq_own_cols[m][:], scalar2=None,
                                op0=AL.add)
                            nc.vector.tensor_tensor(
                                out=negd[:, n * 512:(n + 1) * 512], in0=ps[:],
                                in1=s_chunk[:], op=AL.subtract)

                        mx1 = p_tk.tile([P, 8], f32, tag="mx1", bufs=2)
                        nc.vector.max(mx1[:], negd[:])
                        msk = p_tk.tile([P, N], f32, tag="msk", bufs=1)
                        nc.vector.match_replace(msk[:], mx1[:], negd[:],
                                                NEG_BIG)
                        mx2 = p_tk.tile([P, 8], f32, tag="mx2", bufs=2)
                        nc.vector.max(mx2[:], msk[:])
                        kth = p_tk.tile([P, 1], f32, tag="kth", bufs=2)
                        nc.vector.tensor_copy(kth[:], mx2[:, 1:2])

                        nc.vector.tensor_scalar(
                            out=mask_bf[m][:], in0=negd[:], scalar1=kth[:],
                            scalar2=None, op0=AL.is_ge)
                        nc.vector.reduce_sum(rs1[m][:], mask_bf[m][:],
                                             axis=AX)

                        # transpose mask m, store its AG chunk, fire AllGather
                        for a4 in range(TC // 4):
                            # fp8 transpose-mode requires output element
                            # step 2 in PSUM; evacuate with the same stride.
                            psT = ps_tp.tile([P, 4, P, 2], fp8, tag="tp")
                            for j in range(4):
                                a = a4 * 4 + j
                                nc.tensor.transpose(
                                    psT[:, j, :, 0],
                                    mask_bf[m][:, a * P:(a + 1) * P], id8[:])
                            nc.scalar.copy(
                                mt_sb[:, a4 * 4:(a4 + 1) * 4,
                                      m * P:(m + 1) * P],
                                psT[:, :, :, 0])
                        nc.sync.dma_start(
                            ag1_ins[m][:].rearrange("(a p) q -> p a q", p=P),
                            mt_sb[:, :, m * P:(m + 1) * P])
                        nc.gpsimd.collective_compute(
                            "AllGather", AL.bypass,
                            replica_groups=[list(range(NC_CORES))],
                            ins=[ag1_ins[m][:].opt()],
                            outs=[ag1_outs[m][:].opt()],
                        )

            # ============ phase C: R rows, AllGather #2 ============
            # X[q, j] = M[j, own q] = Mt[own row, j]; Mt row (pid*B+m*128+q),
            # col j=(r*B+c): ag1_out[(r*MC + c//128)*N + row, c%128]
            # Gathered R layout: [m][r][jb][128][512]; one AllGather chunk
            # per m, launched right after R(m) -- overlaps with R(m+1)/Ct.
            # Gathered R layout: per-m tensors [r][jb][128][512]; one
            # AllGather chunk per m, launched right after R(m).
            ag2_ins = [dram.tile([NCH * P, 512], fp8, name=f"ag2i{m}")
                       for m in range(MC)]
            ag2_outs = [dram.tile([NC_CORES * NCH * P, 512], fp8,
                                  addr_space="Shared", name=f"ag2o{m}")
                        for m in range(MC)]
            with tc.tile_pool(name="p_r", bufs=1) as p_r:
                for m in range(MC):
                    rbf = p_r.tile([P, N], fp8, tag="rbf", bufs=2)
                    # X for all sources r, own rows m: one DMA per cb
                    xt = p_r.tile([P, MC, NC_CORES, P], fp8, tag="xt",
                                  bufs=2)
                    for cb in range(MC):
                        nc.sync.dma_start(
                            xt[:, cb, :, :],
                            ag1_outs[cb].rearrange(
                                "(r a) q -> a r q", r=NC_CORES)[
                                bass.ds(pid * B + m * P, P), :, :])
                    for r in range(NC_CORES):
                        nc.vector.tensor_tensor(
                            out=rbf[:, r * B:(r + 1) * B].rearrange(
                                "p (c q) -> p c q", c=MC),
                            in0=mask_bf[m][:, r * B:(r + 1) * B].rearrange(
                                "p (c q) -> p c q", c=MC),
                            in1=xt[:, :, r, :],
                            op=AL.mult)
                    nc.vector.reduce_sum(rsR[m][:], rbf[:], axis=AX)
                    nc.sync.dma_start(
                        ag2_ins[m][:].rearrange("(jb q) j -> q jb j", jb=NCH),
                        rbf.rearrange("p (jb j) -> p jb j", jb=NCH))
                    nc.gpsimd.collective_compute(
                        "AllGather", AL.bypass,
                        replica_groups=[list(range(NC_CORES))],
                        ins=[ag2_ins[m][:].opt()],
                        outs=[ag2_outs[m][:].opt()],
                    )
            cm_mask.__exit__(None, None, None)   # masks dead

            # scale = 0.5 / (rs1 * rsR)
            halfrec = []
            for m in range(MC):
                prod = p_small.tile([P, 1], f32, tag=f"prod{m}",
                                    name=f"prod{m}")
                nc.vector.tensor_tensor(out=prod[:], in0=rs1[m][:],
                                        in1=rsR[m][:], op=AL.mult)
                rec = p_small.tile([P, 1], f32, tag=f"rec{m}",
                                   name=f"rec{m}")
                nc.vector.reciprocal(rec[:], prod[:])
                hr = p_small.tile([P, 1], f32, tag=f"hr{m}", name=f"hr{m}")
                nc.vector.tensor_scalar_mul(hr[:], rec[:], 0.5)
                halfrec.append(hr)

            # ============ phase D: Ct = (M M^T)[:, own] ============
            cm_ct = tc.tile_pool(name="p_ct", bufs=1)              # left
            p_ct = cm_ct.__enter__()
            ct_sb = p_ct.tile([P, TC, B], fp8, tag="ct")            # 32 KB/p
            with tc.tile_pool(name="p_lh", bufs=1) as p_lh:
                # consume AG1 chunks in arrival order (cb outer); two source
                # cores per weight-slab DMA (2 MB contiguous).
                for cb in range(MC):
                    for r2 in range(NC_CORES // 2):
                        lh = p_lh.tile([P, 2, TC, P], fp8, tag="lh", bufs=3)
                        nc.scalar.dma_start(
                            lh[:],
                            ag1_outs[cb][r2 * 2 * N:(r2 * 2 + 2) * N, :]
                            .rearrange("(r a p) q -> p r a q", r=2, p=P))
                        for h in range(2):
                            t = (r2 * 2 + h) * MC + cb
                            ps = ps_mm.tile([P, B], f32, tag="mm")
                            for kc in range(TC):
                                nc.tensor.matmul(
                                    ps[:], lh[:, h, kc, :], mt_sb[:, kc, :],
                                    start=(kc == 0), stop=(kc == TC - 1))
                            nc.scalar.copy(ct_sb[:, t, :], ps[:])
            cm_mt.__exit__(None, None, None)     # mt dead

            # ============ phase E: S2 rows = scale * (Ct^T @ R) ============
            # A2A payload in fp16: the transposed half of the
            # symmetrization carries ~5e-4 relative error on values <= 1,
            # far inside tolerance, and halves the AllToAll bytes. Staged
            # chunk-by-chunk inside the W loop so the AllToAll fires
            # immediately when W finishes.
            a2a_in = dram.tile([N, B], fp16)
            cm_s2 = tc.tile_pool(name="p_s2", bufs=1, side="right")
            p_s2 = cm_s2.__enter__()
            sim2 = [p_s2.tile([P, N], f32, tag=f"sim2_{m}", name=f"sim2_{m}")
                    for m in range(MC)]
            tkc = B // P
            with tc.tile_pool(name="p_w", bufs=1) as p_w:
                for n in range(NCH):
                    rsl = p_w.tile([P, NC_CORES, tkc, 512], fp8,
                                   tag="rsl", bufs=2)
                    for m in range(tkc):
                        nc.sync.dma_start(
                            rsl[:, :, m, :],
                            ag2_outs[m].rearrange(
                                "(r jb q) j -> r jb q j", r=NC_CORES,
                                jb=NCH)[:, n, :, :]
                            .rearrange("r q j -> q r j"))
                    for m in range(MC):
                        ps = ps_mm.tile([P, 512], f32, tag="mm")
                        for t in range(TC):
                            nc.tensor.matmul(
                                ps[:], ct_sb[:, t, m * P:(m + 1) * P],
                                rsl[:, t // tkc, t % tkc, :],
                                start=(t == 0), stop=(t == TC - 1))
                        nc.scalar.activation(
                            sim2[m][:, n * 512:(n + 1) * 512], ps[:],
                            mybir.ActivationFunctionType.Copy,
                            scale=halfrec[m][:])
                        # stage this chunk of the AllToAll payload (fp16)
                        s16 = p_w.tile([P, 512], fp16, tag="s16", bufs=3)
                        nc.vector.tensor_copy(
                            s16[:], sim2[m][:, n * 512:(n + 1) * 512])
                        nc.sync.dma_start(
                            a2a_in[n * B + m * P:n * B + (m + 1) * P, :],
                            s16[:])
            cm_ct.__exit__(None, None, None)     # ct dead

            # ============ phase F: AllToAll + transpose + add ============
            a2a_out = dram.tile([N, B], fp16)
            nc.gpsimd.collective_compute(
                "AllToAll", AL.bypass,
                replica_groups=[list(range(NC_CORES))],
                ins=[a2a_in[:].opt()], outs=[a2a_out[:].opt()],
            )

            with tc.tile_pool(name="p_f", bufs=1) as p_f:
                for a4 in range(TC // 4):
                    recvs = []
                    for j in range(4):
                        a = a4 * 4 + j
                        rv = p_f.tile([P, B], fp16, tag="rv", bufs=8)
                        nc.sync.dma_start(rv[:], a2a_out[a * P:(a + 1) * P, :])
                        recvs.append(rv)
                    for m in range(MC):
                        ps = ps_tp.tile([P, 512], fp16, tag="ps_f", bufs=2)
                        for j in range(4):
                            nc.tensor.transpose(
                                ps[:, j * P:(j + 1) * P],
                                recvs[j][:, m * P:(m + 1) * P], idh[:])
                        ob = p_f.tile([P, 512], f32, tag="ob", bufs=3)
                        nc.vector.tensor_tensor(
                            out=ob[:], in0=ps[:],
                            in1=sim2[m][:, a4 * 512:(a4 + 1) * 512],
                            op=AL.add)
                        nc.sync.dma_start(
                            out_ext[m * P:(m + 1) * P,
                                    a4 * 512:(a4 + 1) * 512],
                            ob[:])
            cm_s2.__exit__(None, None, None)

    nc.compile()
    return nc


def _get_nc():
    if "nc" not in _cache:
        _cache["nc"] = _build()
    return _cache["nc"]


def _bf16(x):
    try:
        import ml_dtypes
        return x.astype(ml_dtypes.bfloat16)
    except ImportError:
        from concourse import mybir
        return x.astype(mybir.dt.np(mybir.dt.bfloat16))


def kernel(z, _profile=False):
    from concourse import bass_utils

    from concourse import mybir
    z = np.ascontiguousarray(np.asarray(z, dtype=np.float32))
    assert z.shape == (N, D), z.shape
    zT = np.ascontiguousarray(z.T)
    eye = np.eye(P, dtype=np.float32)
    id32 = eye
    id16 = _bf16(eye)
    id8 = eye.astype(mybir.dt.np(mybir.dt.float8e4))
    idh = eye.astype(np.float16)

    nc = _get_nc()
    in_maps = []
    for c in range(NC_CORES):
        in_maps.append({
            "zt": zT,
            "zt_own": np.ascontiguousarray(zT[:, c * B:(c + 1) * B]),
            "id32": id32,
            "id16": id16,
            "id8": id8,
            "idh": idh,
        })
    res = bass_utils.run_bass_kernel_spmd(
        nc, in_maps, core_ids=list(range(NC_CORES)), trace=_profile)
    out = np.concatenate(
        [res.results[c]["out"] for c in range(NC_CORES)], axis=0)
    if _profile:
        return out, res
    return out


# revision 23
# speedup vs baseline: 1.3039x; 1.0242x over previous
"""Contextual kNN similarity kernel for Trainium2, 8 NeuronCores.

For z [4096, 512] fp32 computes (matching reference.py's structure):
    d   = sq_i + sq_j - 2 z z^T
    kth = 10th smallest d per row (ties included via <=)
    M   = (d <= kth)                       [N, N] 0/1
    S1  = (M M^T) / rowsum(M)
    R   = M * M^T
    S2  = (S1 @ R^T) / rowsum(R)           (R symmetric)
    out = 0.5 (S2 + S2^T)

Distribution: row-shard over 8 cores (512 rows each, z replicated).
Each core: row block of -d (fp32 PE matmul mirroring the reference's
rounding), local top-10 via DVE max8/match_replace/max8, 0/1 mask in
bf16 (exact). PE-transpose mask -> AllGather (bf16). R rows = M .*
gathered-columns (exact), second AllGather. Then two big matmuls with
exact small-integer arithmetic (bf16 inputs, fp32 PSUM):
  Ct = (M M^T)[:, own]   and   W = sum_t C[t, i] R[t, j]
Both row-normalizations and the final 0.5 fold into one per-row scale
0.5/(rs1*rsR) applied on PSUM evacuation. Symmetrization: AllToAll of
the scaled S2, PE transpose, add. Host only preps/replicates inputs and
concatenates the 8 row-block outputs.
"""
import numpy as np

NC_CORES = 8
N = 4096
D = 512
B = N // NC_CORES        # 512 rows per core
P = 128
MC = B // P              # 4 m-chunks of own rows
NCH = N // 512           # 8 n-chunks
KC = D // P              # 4 contraction chunks for g
TC = N // P              # 32 t/k chunks
NEG_BIG = -3.0e38

_cache = {}


def _build():
    from concourse import bass, bacc, tile, mybir

    f32 = mybir.dt.float32
    bf16 = mybir.dt.bfloat16
    fp8 = mybir.dt.float8e4
    fp16 = mybir.dt.float16
    AL = mybir.AluOpType
    AX = mybir.AxisListType.X

    nc = bacc.Bacc(
        "TRN2",
        target_bir_lowering=False,
        debug=False,
        enable_asserts=False,
        num_devices=NC_CORES,
    )

    zt_in = nc.dram_tensor("zt", [D, N], f32, kind="ExternalInput").ap()
    zt_own_in = nc.dram_tensor("zt_own", [D, B], f32, kind="ExternalInput").ap()
    id32_in = nc.dram_tensor("id32", [P, P], f32, kind="ExternalInput").ap()
    id16_in = nc.dram_tensor("id16", [P, P], bf16, kind="ExternalInput").ap()
    id8_in = nc.dram_tensor("id8", [P, P], fp8, kind="ExternalInput").ap()
    idh_in = nc.dram_tensor("idh", [P, P], fp16, kind="ExternalInput").ap()
    out_ext = nc.dram_tensor("out", [B, N], f32, kind="ExternalOutput").ap()

    with tile.TileContext(nc) as tc:
        with tc.tile_pool(name="ps_mm", bufs=3, space="PSUM") as ps_mm, \
             tc.tile_pool(name="ps_tp", bufs=2, space="PSUM") as ps_tp, \
             tc.tile_pool(name="dram", bufs=1, space="DRAM") as dram, \
             tc.tile_pool(name="p_small", bufs=1) as p_small:

            pid = nc.sync.partition_id()

            ones128 = p_small.tile([P, P], f32, tag="ones128")
            nc.vector.memset(ones128[:], 1.0)
            id32 = p_small.tile([P, P], f32, tag="id32")
            nc.sync.dma_start(id32[:], id32_in[:])
            id16 = p_small.tile([P, P], bf16, tag="id16")
            nc.sync.dma_start(id16[:], id16_in[:])
            id8 = p_small.tile([P, P], fp8, tag="id8")
            nc.sync.dma_start(id8[:], id8_in[:])
            idh = p_small.tile([P, P], fp16, tag="idh")
            nc.sync.dma_start(idh[:], idh_in[:])

            rs1 = [p_small.tile([P, 1], f32, tag=f"rs1_{m}", name=f"rs1_{m}")
                   for m in range(MC)]
            rsR = [p_small.tile([P, 1], f32, tag=f"rsR{m}", name=f"rsR{m}")
                   for m in range(MC)]

            # Overlapping lifetimes (mask A..C, mt B..D, ct D..E, s2 E..F):
            # pools must release in per-side LIFO order, so alternate the
            # chain between the left and right SBUF stacks.
            cm_mask = tc.tile_pool(name="p_mask", bufs=1)          # left
            p_mask = cm_mask.__enter__()
            mask_bf = [p_mask.tile([P, N], fp8, tag=f"mask{m}",
                                   name=f"mask{m}") for m in range(MC)]

            # ===== phase A+B fused: sq, g, negd, top-k, mask, transpose,
            # per-m AllGather chunk (fires as soon as mask m is transposed,
            # overlapping the rest of phase A). zt is streamed in 1 MB slabs
            # to keep SBUF under budget with mt_sb resident.
            cm_mt = tc.tile_pool(name="p_mt", bufs=1, side="right")
            p_mt = cm_mt.__enter__()
            mt_sb = p_mt.tile([P, TC, B], fp8, tag="mt")            # 32 KB/p
            # Gathered mask layout: per-cb tensors [r][N][P].
            ag1_ins = [dram.tile([N, P], fp8, name=f"ag1i{cb}")
                       for cb in range(MC)]
            ag1_outs = [dram.tile([NC_CORES * N, P], fp8,
                                  addr_space="Shared", name=f"ag1o{cb}")
                        for cb in range(MC)]
            zt_v = zt_in.rearrange("(kc p) n -> p kc n", p=P)
            with tc.tile_pool(name="p_a", bufs=1) as p_a:
                zt_own_sb = p_a.tile([P, KC, B], f32, tag="zt_own")
                nc.sync.dma_start(
                    zt_own_sb[:], zt_own_in.rearrange("(kc p) n -> p kc n", p=P))
                sq_bcast = p_a.tile([P, N], f32, tag="sq_bcast")     # 16 KB/p

                # own-rows sq -> per-partition [P,1]; bitwise-equal to
                # sq_bcast values (same systolic + psum accumulation order)
                sq_own_cols = []
                with tc.tile_pool(name="p_zsq", bufs=1) as p_zsq:
                    zsq_own = p_zsq.tile([P, KC, B], f32, tag="zsq_own")
                    for kc in range(KC):
                        nc.vector.tensor_mul(
                            zsq_own[:, kc, :], zt_own_sb[:, kc, :],
                            zt_own_sb[:, kc, :])
                    for m in range(MC):
                        ps = ps_mm.tile([P, P], f32, tag="mm")
                        for kc in range(KC):
                            nc.tensor.matmul(
                                ps[:], ones128[:],
                                zsq_own[:, kc, m * P:(m + 1) * P],
                                start=(kc == 0), stop=(kc == KC - 1))
                        sq_bc_m = p_zsq.tile([P, P], f32, tag="sq_bc_m",
                                             bufs=2)
                        nc.scalar.copy(sq_bc_m[:], ps[:])
                        psT = ps_tp.tile([P, P], f32, tag="tp")
                        nc.tensor.transpose(psT[:], sq_bc_m[:], id32[:])
                        sqc = p_small.tile([P, 1], f32, tag=f"sq_own{m}",
                                           name=f"sq_own{m}")
                        nc.vector.tensor_copy(sqc[:], psT[:, 0:1])
                        sq_own_cols.append(sqc)

                # lhsT for g: 2 * zt_own (exact)
                zt_own2 = p_a.tile([P, KC, B], f32, tag="zt_own2")
                for kc in range(KC):
                    nc.vector.tensor_scalar_mul(
                        zt_own2[:, kc, :], zt_own_sb[:, kc, :], 2.0)

                with tc.tile_pool(name="p_tk", bufs=1) as p_tk:
                    for m in range(MC):
                        negd = p_tk.tile([P, N], f32, tag="negd", bufs=2)
                        for n in range(NCH):
                            slab = p_tk.tile([P, KC, 512], f32, tag="slab",
                                             bufs=3)
                            nc.sync.dma_start(
                                slab[:], zt_v[:, :, n * 512:(n + 1) * 512])
                            if m == 0:
                                # sq_bcast[n] (exact, fixed order)
                                zq = p_tk.tile([P, KC, 512], f32, tag="zq",
                                               bufs=2)
                                for kc in range(KC):
                                    nc.vector.tensor_mul(
                                        zq[:, kc, :], slab[:, kc, :],
                                        slab[:, kc, :])
                                psq = ps_mm.tile([P, 512], f32, tag="mm")
                                for kc in range(KC):
                                    nc.tensor.matmul(
                                        psq[:], ones128[:], zq[:, kc, :],
                                        start=(kc == 0), stop=(kc == KC - 1))
                                nc.scalar.copy(
                                    sq_bcast[:, n * 512:(n + 1) * 512],
                                    psq[:])
                            ps = ps_mm.tile([P, 512], f32, tag="mm")
                            for kc in range(KC):
                                nc.tensor.matmul(
                                    ps[:],
                                    zt_own2[:, kc, m * P:(m + 1) * P],
                                    slab[:, kc, :],
                                    start=(kc == 0), stop=(kc == KC - 1))
                            s_chunk = p_tk.tile([P, 512], f32, tag="s_chunk",
                                                bufs=2)
                            nc.vector.tensor_scalar(
                                out=s_chunk[:],
                                in0=sq_bcast[:, n * 512:(n + 1) * 512],
                                scalar1=sq_own_cols[m][:], scalar2=None,
                                op0=AL.add)
                            nc.vector.tensor_tensor(
                                out=negd[:, n * 512:(n + 1) * 512], in0=ps[:],
                                in1=s_chunk[:], op=AL.subtract)

                        mx1 = p_tk.tile([P, 8], f32, tag="mx1", bufs=2)
                        nc.vector.max(mx1[:], negd[:])
                        msk = p_tk.tile([P, N], f32, tag="msk", bufs=1)
                        nc.vector.match_replace(msk[:], mx1[:], negd[:],
                                                NEG_BIG)
                        mx2 = p_tk.tile([P, 8], f32, tag="mx2", bufs=2)
                        nc.vector.max(mx2[:], msk[:])
                        kth = p_tk.tile([P, 1], f32, tag="kth", bufs=2)
                        nc.vector.tensor_copy(kth[:], mx2[:, 1:2])

                        nc.vector.tensor_scalar(
                            out=mask_bf[m][:], in0=negd[:], scalar1=kth[:],
                            scalar2=None, op0=AL.is_ge)
                        nc.vector.reduce_sum(rs1[m][:], mask_bf[m][:],
                                             axis=AX)

                        # transpose mask m, store its AG chunk, fire AllGather
                        for a4 in range(TC // 4):
                            # fp8 transpose-mode requires output element
                            # step 2 in PSUM; evacuate with the same stride.
                            psT = ps_tp.tile([P, 4, P, 2], fp8, tag="tp")
                            for j in range(4):
                                a = a4 * 4 + j
                                nc.tensor.transpose(
                                    psT[:, j, :, 0],
                                    mask_bf[m][:, a * P:(a + 1) * P], id8[:])
                            nc.scalar.copy(
                                mt_sb[:, a4 * 4:(a4 + 1) * 4,
                                      m * P:(m + 1) * P],
                                psT[:, :, :, 0])
                        nc.gpsimd.dma_start(
                            ag1_ins[m][:].rearrange("(a p) q -> p a q", p=P),
                            mt_sb[:, :, m * P:(m + 1) * P])
                        nc.gpsimd.collective_compute(
                            "AllGather", AL.bypass,
                            replica_groups=[list(range(NC_CORES))],
                            ins=[ag1_ins[m][:].opt()],
                            outs=[ag1_outs[m][:].opt()],
                        )

            # ---- mask AllToAll: delivers X = Mt[own rows, :] without
            # waiting for the chained AllGathers. Shard d of the payload is
            # mt_sb rows k in d's block; receiving and stacking by source
            # gives X[q, j] = Mt[own_start+q, j] directly.
            a2am_in = dram.tile([N, B], fp8)
            nc.gpsimd.dma_start(
                a2am_in[:].rearrange("(a p) j -> p a j", p=P), mt_sb[:])
            a2am_out = dram.tile([N, B], fp8)
            nc.gpsimd.collective_compute(
                "AllToAll", AL.bypass,
                replica_groups=[list(range(NC_CORES))],
                ins=[a2am_in[:].opt()], outs=[a2am_out[:].opt()],
            )

            # ============ phase C: R rows, AllGather #2 ============
            # X[q, j] = M[j, own q] = Mt[own row, j]; Mt row (pid*B+m*128+q),
            # col j=(r*B+c): ag1_out[(r*MC + c//128)*N + row, c%128]
            # Gathered R layout: [m][r][jb][128][512]; one AllGather chunk
            # per m, launched right after R(m) -- overlaps with R(m+1)/Ct.
            # Gathered R layout: per-m tensors [r][jb][128][512]; one
            # AllGather chunk per m, launched right after R(m).
            ag2_ins = [dram.tile([NCH * P, 512], fp8, name=f"ag2i{m}")
                       for m in range(MC)]
            ag2_outs = [dram.tile([NC_CORES * NCH * P, 512], fp8,
                                  addr_space="Shared", name=f"ag2o{m}")
                        for m in range(MC)]
            with tc.tile_pool(name="p_r", bufs=1) as p_r:
                for m in range(MC):
                    rbf = p_r.tile([P, N], fp8, tag="rbf", bufs=2)
                    xt = p_r.tile([P, NC_CORES, B], fp8, tag="xt", bufs=2)
                    nc.sync.dma_start(
                        xt[:],
                        a2am_out.rearrange("(r s) j -> r s j", r=NC_CORES)[
                            :, m * P:(m + 1) * P, :]
                        .rearrange("r s j -> s r j"))
                    for r in range(NC_CORES):
                        nc.vector.tensor_tensor(
                            out=rbf[:, r * B:(r + 1) * B],
                            in0=mask_bf[m][:, r * B:(r + 1) * B],
                            in1=xt[:, r, :],
                            op=AL.mult)
                    nc.vector.reduce_sum(rsR[m][:], rbf[:], axis=AX)
                    nc.gpsimd.dma_start(
                        ag2_ins[m][:].rearrange("(jb q) j -> q jb j", jb=NCH),
                        rbf.rearrange("p (jb j) -> p jb j", jb=NCH))
                    nc.gpsimd.collective_compute(
                        "AllGather", AL.bypass,
                        replica_groups=[list(range(NC_CORES))],
                        ins=[ag2_ins[m][:].opt()],
                        outs=[ag2_outs[m][:].opt()],
                    )
            cm_mask.__exit__(None, None, None)   # masks dead

            # scale = 0.5 / (rs1 * rsR)
            halfrec = []
            for m in range(MC):
                prod = p_small.tile([P, 1], f32, tag=f"prod{m}",
                                    name=f"prod{m}")
                nc.vector.tensor_tensor(out=prod[:], in0=rs1[m][:],
                                        in1=rsR[m][:], op=AL.mult)
                rec = p_small.tile([P, 1], f32, tag=f"rec{m}",
                                   name=f"rec{m}")
                nc.vector.reciprocal(rec[:], prod[:])
                hr = p_small.tile([P, 1], f32, tag=f"hr{m}", name=f"hr{m}")
                nc.vector.tensor_scalar_mul(hr[:], rec[:], 0.5)
                halfrec.append(hr)

            # ============ phase D: Ct = (M M^T)[:, own] ============
            cm_ct = tc.tile_pool(name="p_ct", bufs=1)              # left
            p_ct = cm_ct.__enter__()
            ct_sb = p_ct.tile([P, TC, B], fp8, tag="ct")            # 32 KB/p
            with tc.tile_pool(name="p_lh", bufs=1) as p_lh:
                # consume AG1 chunks in arrival order (cb outer); two source
                # cores per weight-slab DMA (2 MB contiguous).
                for cb in range(MC):
                    for r2 in range(NC_CORES // 2):
                        lh = p_lh.tile([P, 2, TC, P], fp8, tag="lh", bufs=3)
                        nc.scalar.dma_start(
                            lh[:],
                            ag1_outs[cb][r2 * 2 * N:(r2 * 2 + 2) * N, :]
                            .rearrange("(r a p) q -> p r a q", r=2, p=P))
                        for h in range(2):
                            t = (r2 * 2 + h) * MC + cb
                            ps = ps_mm.tile([P, B], f32, tag="mm")
                            for kc in range(TC):
                                nc.tensor.matmul(
                                    ps[:], lh[:, h, kc, :], mt_sb[:, kc, :],
                                    start=(kc == 0), stop=(kc == TC - 1))
                            nc.scalar.copy(ct_sb[:, t, :], ps[:])
            cm_mt.__exit__(None, None, None)     # mt dead

            # ============ phase E: S2 rows = scale * (Ct^T @ R) ============
            # A2A payload in fp16: the transposed half of the
            # symmetrization carries ~5e-4 relative error on values <= 1,
            # far inside tolerance, and halves the AllToAll bytes. Staged
            # chunk-by-chunk inside the W loop so the AllToAll fires
            # immediately when W finishes.
            a2a_in = dram.tile([N, B], fp16)
            cm_s2 = tc.tile_pool(name="p_s2", bufs=1, side="right")
            p_s2 = cm_s2.__enter__()
            sim2 = [p_s2.tile([P, N], f32, tag=f"sim2_{m}", name=f"sim2_{m}")
                    for m in range(MC)]
            tkc = B // P
            with tc.tile_pool(name="p_w", bufs=1) as p_w:
                for n in range(NCH):
                    rsl = p_w.tile([P, NC_CORES, tkc, 512], fp8,
                                   tag="rsl", bufs=2)
                    for m in range(tkc):
                        nc.sync.dma_start(
                            rsl[:, :, m, :],
                            ag2_outs[m].rearrange(
                                "(r jb q) j -> r jb q j", r=NC_CORES,
                                jb=NCH)[:, n, :, :]
                            .rearrange("r q j -> q r j"))
                    for m in range(MC):
                        ps = ps_mm.tile([P, 512], f32, tag="mm")
                        for t in range(TC):
                            nc.tensor.matmul(
                                ps[:], ct_sb[:, t, m * P:(m + 1) * P],
                                rsl[:, t // tkc, t % tkc, :],
                                start=(t == 0), stop=(t == TC - 1))
                        nc.scalar.activation(
                            sim2[m][:, n * 512:(n + 1) * 512], ps[:],
                            mybir.ActivationFunctionType.Copy,
                            scale=halfrec[m][:])
                        # stage this chunk of the AllToAll payload (fp16)
                        s16 = p_w.tile([P, 512], fp16, tag="s16", bufs=3)
                        nc.vector.tensor_copy(
                            s16[:], sim2[m][:, n * 512:(n + 1) * 512])
                        for rr in range(max(1, 512 // B)):
                            r = (n * 512) // B + rr
                            nc.sync.dma_start(
                                a2a_in[r * B + m * P:r * B + (m + 1) * P, :],
                                s16[:, rr * B:rr * B + min(B, 512)])
            cm_ct.__exit__(None, None, None)     # ct dead

            # ============ phase F: AllToAll + transpose + add ============
            a2a_out = dram.tile([N, B], fp16)
            nc.gpsimd.collective_compute(
                "AllToAll", AL.bypass,
                replica_groups=[list(range(NC_CORES))],
                ins=[a2a_in[:].opt()], outs=[a2a_out[:].opt()],
            )

            with tc.tile_pool(name="p_f", bufs=1) as p_f:
                for a4 in range(TC // 4):
                    recvs = []
                    for j in range(4):
                        a = a4 * 4 + j
                        rv = p_f.tile([P, B], fp16, tag="rv", bufs=8)
                        nc.sync.dma_start(rv[:], a2a_out[a * P:(a + 1) * P, :])
                        recvs.append(rv)
                    for m in range(MC):
                        ps = ps_tp.tile([P, 512], fp16, tag="ps_f", bufs=2)
                        for j in range(4):
                            nc.tensor.transpose(
                                ps[:, j * P:(j + 1) * P],
                                recvs[j][:, m * P:(m + 1) * P], idh[:])
                        ob = p_f.tile([P, 512], f32, tag="ob", bufs=3)
                        nc.vector.tensor_tensor(
                            out=ob[:], in0=ps[:],
                            in1=sim2[m][:, a4 * 512:(a4 + 1) * 512],
                            op=AL.add)
                        nc.sync.dma_start(
                            out_ext[m * P:(m + 1) * P,
                                    a4 * 512:(a4 + 1) * 512],
                            ob[:])
            cm_s2.__exit__(None, None, None)

    nc.compile()
    return nc


def _get_nc():
    if "nc" not in _cache:
        _cache["nc"] = _build()
    return _cache["nc"]


def _bf16(x):
    try:
        import ml_dtypes
        return x.astype(ml_dtypes.bfloat16)
    except ImportError:
        from concourse import mybir
        return x.astype(mybir.dt.np(mybir.dt.bfloat16))


def kernel(z, _profile=False):
    from concourse import bass_utils

    from concourse import mybir
    z = np.ascontiguousarray(np.asarray(z, dtype=np.float32))
    assert z.shape == (N, D), z.shape
    zT = np.ascontiguousarray(z.T)
    eye = np.eye(P, dtype=np.float32)
    id32 = eye
    id16 = _bf16(eye)
    id8 = eye.astype(mybir.dt.np(mybir.dt.float8e4))
    idh = eye.astype(np.float16)

    nc = _get_nc()
    in_maps = []
    for c in range(NC_CORES):
        in_maps.append({
            "zt": zT,
            "zt_own": np.ascontiguousarray(zT[:, c * B:(c + 1) * B]),
            "id32": id32,
            "id16": id16,
            "id8": id8,
            "idh": idh,
        })
    res = bass_utils.run_bass_kernel_spmd(
        nc, in_maps, core_ids=list(range(NC_CORES)), trace=_profile)
    out = np.concatenate(
        [res.results[c]["out"] for c in range(NC_CORES)], axis=0)
    if _profile:
        return out, res
    return out
